# revision 34
# baseline (speedup 1.0000x reference)
"""Trainium2 Bass kernel: gumbel-softmax-argmax embedding lookup (end-to-end).

Reference math (nn_End2End_49495203119139):
    hot  = argmax_V(softmax((logits + gumbel)/tau))       == argmax_V(logits+gumbel)
    row  = grid_sample-nearest index map of hot            == ROWMAP[hot]  (LUT)
    tok_emb = W[row][:, col_map]   with col_map == arange(E)  (verified at runtime)
    inputs_embeds = tok_emb * mask
    psg_roll = roll(psg_ids, 1, axis=1); psg_roll[:,0] = 1
    extr  = (1 - mask[:, ::-1]) * psg_roll
    trunc = rotate_right(extr, shifts) with shifts = mask.sum(-1)   (per row)
    flag  = cumsum(trunc != 0, -1) > 0
    out   = inputs_embeds + where(flag, W[trunc], 0)

Key observation: inputs_embeds is multiplied by the attention mask, so the
argmax over the 32128-wide vocab -- the part that forces streaming the
526 MB of logits+gumbel -- is only needed for tokens with mask == 1.
The mask is an input, so sharding can compact: the host flattens the
active (b, l) positions (N of B*L), pads to C = ceil(N/8) per core, and
ships each core a compacted [C, V] slice of logits and gumbel.  Each core
streams only its C rows (vs B*L/8 = 256 for dense data-parallel).  The
passage branch (cheap) stays on the owner core of each batch row; the two
partial outputs (psg_out for all positions, tok_out rows for the active
tokens) are combined by the host during the unshard step -- an exact f32
add of the same values the dense kernel would have added on-device.

Per-core device plan (memory-bound part = streaming the compacted
logits+gumbel, 2*C*V*4 bytes ~= 45.8 MB for C=178, ~127 us HBM floor at
360 GB/s per core).  Rows go in two partition groups, (C-128, 128); per
group and per vocab span: HWDGE-load the logits span (alternating SP/ACT
DMA queues), add the gumbel span with one SWDGE inline-accumulate DMA
(CCE add; descriptors must stay <= 2048 elements), then find the span
max on DVE.  The argmax strategy differs per group because DVE is the
scarce engine after compaction (max+max_index of every span would put
~145 us on DVE vs the ~135 us DMA floor):
  - FIRST group (C-128 rows): max-only streaming -- one reduce_max per
    span at 251-element sub-span granularity.  A short tournament picks
    the LOWEST sub-span attaining the global max (argmax tie rule), then
    that sub-span alone is re-read (indirect row gather + CCE-add) and
    ONE max_index against the broadcast global max finds the in-span
    position -- exact because the CCE re-add bit-matches the stream
    pass.  Its tail hides under the second group's stream; the max_index
    is deferred a few spans into it so the re-read's DMA latency never
    blocks the in-order DVE queue.
  - LAST group (128 rows; carries the terminal tail): per-span max +
    max_index inline (affordable: its 5.7 us/span DMA outpaces the
    4.4 us/span DVE cost), so after the final span only the tournament
    and the embedding gather remain.  The vocab end is tapered into
    shrinking spans so the final span's DVE work is short.
  - hot -> token embedding via ONE indirect gather from a host-built
    table W_pre = W[rowmap][:, colmap] (constant-folds the grid_sample
    LUTs, removing a dependent gather from the critical tail).
  - the passage branch is pure index arithmetic on [128,1] tiles: the
    reverse/roll/rotate are folded into gather indices modulo L, the
    mask-sum and cumsum are 0/1 matmuls against ones/triangular matrices
    (exact in any PE precision).  psg_out = flag * W[trunc] is stored for
    every position.  It is emitted in three stages (index math + row
    gathers up front, combine after the first group's stream, flag+store
    mid second stream) so its cross-engine waits never stall the DVE
    queue ahead of stream work.
"""

import numpy as np

B = 16
L = 128
V = 32128
E = 768
N_CORES = 8
CH = 2008                     # vocab chunk (free dim) per streamed tile;
                              # <= 2048 elements so a gumbel chunk is ONE
                              # CCE-add DMA descriptor per partition row
NCH = V // CH                 # 16 chunks


def _build_v2(nc_mod, cap, dims=None):
    """Per-core module for the compacted layout. cap = tokens per core."""
    import concourse.tile as tile
    from concourse import bass, mybir
    from concourse.bass import IndirectOffsetOnAxis

    d = dims or {}
    ch = d.get("CH", CH)
    nch = V // ch
    lbufs = d.get("LBUFS", 12)
    # rescan-mode groups track chunk maxes at sub-chunk granularity so the
    # winning-chunk re-read (and its max_index) touch only SW elements
    SUB = ch // 251
    sw = 251                      # V == nch*SUB*sw exactly
    nfine = nch * SUB             # 128 fine slots
    # maxidx-mode groups taper the end of the vocab so the last spans'
    # (max + max_index) DVE work after the final CCE-add is short
    taper = d.get("TAPER", (1255, 1255, 1255, 251) if ch == 2008 else
                  (502, 502, 502, 502))
    nfull = nch - sum(taper) // ch
    assert sum(taper) % ch == 0
    spans_mi = [(c * ch, ch) for c in range(nfull)]
    tail_lo = nfull * ch
    for w_ in taper:
        spans_mi.append((tail_lo, w_))
        tail_lo += w_
    assert tail_lo == V
    nsp = len(spans_mi)

    nc = nc_mod
    f32 = mybir.dt.float32
    i32 = mybir.dt.int32
    u32 = mybir.dt.uint32
    Op = mybir.AluOpType
    AX = mybir.AxisListType

    # partition groups covering the cap rows.  The LAST group carries the
    # terminal tail, so it uses baseline per-chunk max_index (short tail, no
    # re-read) -- affordable only when its DMA per chunk (rows*ch*8B/360GBps)
    # outpaces the ~4.4us/chunk of DVE work, i.e. rows >= ~100.  Small groups
    # use max-only streaming with a winning-chunk re-read; their longer tail
    # hides under the next group's stream.
    groups = []  # (row0, rows, mode)
    g1mode = d.get("G1MODE", "maxidx")
    if cap > 128:
        g1rows = min(d.get("G1ROWS", 128), cap)
        g0rows = cap - g1rows
        if g0rows < 48:
            # very small streaming groups crashed the device (validated down
            # to 48 rows); rebalance and drop to rescan mode if the terminal
            # group gets too small for inline max_index to keep DVE fed
            g0rows = 48
            g1rows = cap - g0rows
        mode1 = g1mode if g1rows >= 100 else "rescan"
        groups.append((0, g0rows, "rescan"))
        groups.append((g0rows, g1rows, mode1))
    else:
        groups.append((0, cap, g1mode if cap >= 128 else "rescan"))

    lg_h = nc.dram_tensor("lg", [cap, V], f32, kind="ExternalInput")
    gm_h = nc.dram_tensor("gm", [cap, V], f32, kind="ExternalInput")
    mask_h = nc.dram_tensor("mask", [2 * L, 1], i32, kind="ExternalInput")
    psg_h = nc.dram_tensor("psg", [2 * L, 1], i32, kind="ExternalInput")
    wte_h = nc.dram_tensor("wte", [V, E], f32, kind="ExternalInput")
    wpre_h = nc.dram_tensor("wpre", [V, E], f32, kind="ExternalInput")
    tri_h = nc.dram_tensor("tri", [L, L], f32, kind="ExternalInput")
    psgout_h = nc.dram_tensor("psg_out", [2 * L, E], f32, kind="ExternalOutput")
    tokout_h = nc.dram_tensor("tok_out", [cap, E], f32, kind="ExternalOutput")

    # fine-sliced row views for the data-dependent winning-slot re-read
    lg_v = lg_h[:].rearrange("r (c w) -> (r c) w", w=sw)
    gm_v = gm_h[:].rearrange("r (c w) -> (r c) w", w=sw)

    with tile.TileContext(nc) as tc:
        with (
            tc.tile_pool(name="lpool", bufs=lbufs) as lpool,
            tc.tile_pool(name="lpool1", bufs=d.get("LBUFS1", 5)) as lpool1,
            tc.tile_pool(name="stats", bufs=2) as stats,
            tc.tile_pool(name="small", bufs=2) as small,
            tc.tile_pool(name="emb", bufs=2) as emb,
            tc.tile_pool(name="consts", bufs=1) as consts,
            tc.tile_pool(name="psum", bufs=2, space="PSUM") as psum,
        ):
            # ---- per-core constants (built once) ----
            ones_mat = consts.tile([L, L], f32)
            nc.vector.memset(ones_mat[:], 1.0)
            # tri rides the SWDGE queue: Pool's first CCE descriptor-gen
            # waits for the chunk-0 load anyway, so this slots in for free
            tri_sb = consts.tile([L, L], f32)
            nc.gpsimd.dma_start(out=tri_sb[:], in_=tri_h[:])

            iota_p_i = consts.tile([L, 1], i32)
            nc.gpsimd.iota(iota_p_i[:], pattern=[[1, 1]], base=0, channel_multiplier=1)
            iota_p = consts.tile([L, 1], f32)
            nc.vector.tensor_copy(out=iota_p[:], in_=iota_p_i[:])

            need_mi = any(m == "maxidx" for (_, _, m) in groups)
            need_rs = any(m == "rescan" for (_, _, m) in groups)
            if need_mi:
                iotas_i = consts.tile([L, nsp], i32)
                nc.gpsimd.iota(iotas_i[:], pattern=[[1, nsp]], base=0, channel_multiplier=0)
                iotas = consts.tile([L, nsp], f32)
                nc.vector.tensor_copy(out=iotas[:], in_=iotas_i[:])
                # srev[c] = nsp - c (used to pick the LOWEST span attaining the max)
                srev = consts.tile([L, nsp], f32)
                nc.vector.tensor_scalar(srev[:], iotas[:], -1.0, float(nsp), op0=Op.mult, op1=Op.add)
                # per-span start offsets (hot = bases[c*] + within-span index)
                bases = consts.tile([L, nsp], f32)
                nc.vector.tensor_scalar(bases[:], iotas[:], float(ch), None, op0=Op.mult)
                for ci, (lo_c, _w) in enumerate(spans_mi):
                    if lo_c != ci * ch:
                        nc.vector.memset(bases[:, ci:ci + 1], float(lo_c))
            if need_rs:
                iotaf_i = consts.tile([L, nfine], i32)
                nc.gpsimd.iota(iotaf_i[:], pattern=[[1, nfine]], base=0, channel_multiplier=0)
                iotaf = consts.tile([L, nfine], f32)
                nc.vector.tensor_copy(out=iotaf[:], in_=iotaf_i[:])
                frev = consts.tile([L, nfine], f32)
                nc.vector.tensor_scalar(frev[:], iotaf[:], -1.0, float(nfine), op0=Op.mult, op1=Op.add)
            # per-partition fine-sliced row base: (row + group offset) * nfine
            rowb = {}
            for (g0_, _gp, mode) in groups:
                if mode != "rescan":
                    continue
                rb = consts.tile([L, 1], f32)
                nc.vector.tensor_scalar(rb[:], iota_p[:], float(nfine), float(g0_ * nfine),
                                        op0=Op.mult, op1=Op.add)
                rowb[g0_] = rb

            psg_state = {}

            def psg_a(t):
                """psg stage 1: mask load, length, gather indices; issues the
                mask/psg row gathers.  All DVE ops here depend only on the
                mask load + ones matmul, so they run during kernel startup."""
                tok = slice(t * L, (t + 1) * L)
                mask_i = small.tile([L, 1], i32, tag="mask_i")
                nc.scalar.dma_start(out=mask_i[:], in_=mask_h[tok, :])
                mask_f = small.tile([L, 1], f32, tag="mask_f")
                nc.vector.tensor_copy(out=mask_f[:], in_=mask_i[:])

                # s (broadcast to all partitions) = sum(mask) via ones matmul
                s_ps = psum.tile([L, 1], f32, tag="s_ps")
                nc.tensor.matmul(out=s_ps[:], lhsT=ones_mat[:], rhs=mask_f[:], start=True, stop=True)
                psg_state[t] = dict(s_ps=s_ps)

            def psg_a2(t):
                """psg stage 1b: index arithmetic (the cold-PE matmul from
                stage 1a is long done, so nothing here stalls the DVE queue);
                issues the mask/psg row gathers."""
                st = psg_state[t]
                s_ps = st["s_ps"]
                s_bc = small.tile([L, 1], f32, tag="s_bc")
                nc.vector.tensor_copy(out=s_bc[:], in_=s_ps[:])

                def mod_l(x_ap, lo_fix=True, hi_fix=True, tagp=""):
                    # x <- x mod L for x in (-L, 2L)
                    if hi_fix:
                        ge = small.tile([L, 1], f32, tag="ge" + tagp)
                        nc.vector.tensor_scalar(ge[:], x_ap, float(L), None, op0=Op.is_ge)
                        nc.vector.scalar_tensor_tensor(
                            out=x_ap, in0=ge[:], scalar=-float(L), in1=x_ap, op0=Op.mult, op1=Op.add)
                    if lo_fix:
                        lt_ = small.tile([L, 1], f32, tag="lt" + tagp)
                        nc.vector.tensor_scalar(lt_[:], x_ap, 0.0, None, op0=Op.is_lt)
                        nc.vector.scalar_tensor_tensor(
                            out=x_ap, in0=lt_[:], scalar=float(L), in1=x_ap, op0=Op.mult, op1=Op.add)

                # fidx = (L-1 + s - l) mod L   (flipped-mask gather index)
                fidx = small.tile([L, 1], f32, tag="fidx")
                nc.vector.scalar_tensor_tensor(
                    out=fidx[:], in0=s_bc[:], scalar=float(L - 1), in1=iota_p[:],
                    op0=Op.add, op1=Op.subtract)
                mod_l(fidx[:], lo_fix=False, tagp="f")

                # pidx = (L-1 - s + l) mod L   (rolled-psg gather index)
                pidx = small.tile([L, 1], f32, tag="pidx")
                nc.vector.scalar_tensor_tensor(
                    out=pidx[:], in0=s_bc[:], scalar=-1.0, in1=iota_p[:],
                    op0=Op.mult, op1=Op.add)
                nc.vector.tensor_scalar(pidx[:], pidx[:], float(L - 1), None, op0=Op.add)
                mod_l(pidx[:], tagp="p")

                # k = (l - s) mod L ; BOS position is k == 0
                kk = small.tile([L, 1], f32, tag="kk")
                nc.vector.scalar_tensor_tensor(
                    out=kk[:], in0=s_bc[:], scalar=-1.0, in1=iota_p[:],
                    op0=Op.mult, op1=Op.add)
                mod_l(kk[:], hi_fix=False, tagp="k")
                bos = small.tile([L, 1], f32, tag="bos")
                nc.vector.tensor_scalar(bos[:], kk[:], 0.0, None, op0=Op.is_equal)

                # gather mask[fidx] and psg[pidx] (within this batch row)
                fr_i = small.tile([L, 1], i32, tag="fr_i")
                nc.vector.tensor_scalar(fidx[:], fidx[:], float(t * L), None, op0=Op.add)
                nc.vector.tensor_copy(out=fr_i[:], in_=fidx[:])
                mrev = small.tile([L, 1], i32, tag="mrev")
                nc.gpsimd.indirect_dma_start(
                    out=mrev[:], out_offset=None, in_=mask_h[:],
                    in_offset=IndirectOffsetOnAxis(ap=fr_i[:, 0:1], axis=0),
                )
                pr_i = small.tile([L, 1], i32, tag="pr_i")
                nc.vector.tensor_scalar(pidx[:], pidx[:], float(t * L), None, op0=Op.add)
                nc.vector.tensor_copy(out=pr_i[:], in_=pidx[:])
                prot = small.tile([L, 1], i32, tag="prot")
                nc.gpsimd.indirect_dma_start(
                    out=prot[:], out_offset=None, in_=psg_h[:],
                    in_offset=IndirectOffsetOnAxis(ap=pr_i[:, 0:1], axis=0),
                )
                st.update(bos=bos, mrev=mrev, prot=prot)

            def psg_b(t):
                """psg stage 2: combine the gathered rows (long since arrived)
                into trunc/flag; issues the psg embedding gather."""
                st = psg_state[t]
                bos, mrev, prot = st["bos"], st["mrev"], st["prot"]
                # f_rot = 1 - mask[fidx]
                mrev_f = small.tile([L, 1], f32, tag="mrev_f")
                nc.vector.tensor_copy(out=mrev_f[:], in_=mrev[:])
                frot = small.tile([L, 1], f32, tag="frot")
                nc.vector.tensor_scalar(frot[:], mrev_f[:], -1.0, 1.0, op0=Op.mult, op1=Op.add)
                # psg_rot = bos ? 1 : psg[pidx]
                prot_f = small.tile([L, 1], f32, tag="prot_f")
                nc.vector.tensor_copy(out=prot_f[:], in_=prot[:])
                nbos = small.tile([L, 1], f32, tag="nbos")
                nc.vector.tensor_scalar(nbos[:], bos[:], -1.0, 1.0, op0=Op.mult, op1=Op.add)
                nc.vector.tensor_tensor(out=prot_f[:], in0=prot_f[:], in1=nbos[:], op=Op.mult)
                nc.vector.tensor_tensor(out=prot_f[:], in0=prot_f[:], in1=bos[:], op=Op.add)
                # trunc = f_rot * psg_rot
                trunc = small.tile([L, 1], f32, tag="trunc")
                nc.vector.tensor_tensor(out=trunc[:], in0=frot[:], in1=prot_f[:], op=Op.mult)

                # flag = cumsum(trunc != 0) > 0 via triangular matmul
                nz = small.tile([L, 1], f32, tag="nz")
                nc.vector.tensor_scalar(nz[:], trunc[:], 0.0, None, op0=Op.not_equal)
                cnt_ps = psum.tile([L, 1], f32, tag="cnt_ps")
                nc.tensor.matmul(out=cnt_ps[:], lhsT=tri_sb[:], rhs=nz[:], start=True, stop=True)

                trunc_i = small.tile([L, 1], i32, tag="trunc_i")
                nc.vector.tensor_copy(out=trunc_i[:], in_=trunc[:])
                psgemb = emb.tile([L, E], f32, tag="psgemb")
                nc.gpsimd.indirect_dma_start(
                    out=psgemb[:], out_offset=None, in_=wte_h[:],
                    in_offset=IndirectOffsetOnAxis(ap=trunc_i[:, 0:1], axis=0),
                )
                st["cnt_ps"] = cnt_ps
                st["psgemb"] = psgemb

            def psg_c(t):
                """psg stage 3: flag-mask the embeddings and store."""
                st = psg_state[t]
                tok = slice(t * L, (t + 1) * L)
                flag = small.tile([L, 1], f32, tag="flag")
                nc.vector.tensor_scalar(flag[:], st["cnt_ps"][:], 0.0, None, op0=Op.is_gt)
                outp = emb.tile([L, E], f32, tag="outp")
                nc.vector.tensor_scalar(outp[:], st["psgemb"][:], flag[:, 0:1],
                                        None, op0=Op.mult)
                nc.sync.dma_start(out=psgout_h[tok, :], in_=outp[:])

            def stream_group(g0_, gp, mode, interleave=None):
                """DMA-bound pass over the vocab for rows [g0_, g0_+gp):
                per span, load logits, accumulate gumbel in the DMA
                datapath, then track span maxes (and, in maxidx mode, the
                first within-span argmax position).  `interleave` is the
                prior group's deferred rescan finish, emitted a few chunks
                in -- by then its re-read data has arrived, so the max_index
                never stalls the in-order DVE queue."""
                rows = slice(g0_, g0_ + gp)
                idx_all = None
                if mode == "maxidx":
                    m_all = stats.tile([L, nsp], f32, tag="m_all" + mode)
                    idx_all = stats.tile([L, nsp], f32, tag="idx_all")
                    spans = spans_mi
                else:
                    m_all = stats.tile([L, nfine], f32, tag="m_all" + mode)
                    spans = [(c * ch, ch) for c in range(nch)]
                pool = lpool if g0_ == 0 else lpool1
                for c, (lo, w) in enumerate(spans):
                    for fn in (interleave or {}).get(c, []):
                        fn()
                    lt = pool.tile([L, ch], f32, tag="lt")
                    ldeng = nc.scalar if c % 2 else nc.sync
                    ldeng.dma_start(out=lt[0:gp, 0:w], in_=lg_h[rows, lo:lo + w])
                    nc.gpsimd.dma_start(
                        out=lt[0:gp, 0:w], in_=gm_h[rows, lo:lo + w], accum_op=Op.add)
                    if mode == "maxidx":
                        mx8 = small.tile([L, 8], f32, tag="mx8")
                        nc.vector.max(out=mx8[0:gp, :], in_=lt[0:gp, 0:w])
                        nc.vector.tensor_copy(out=m_all[0:gp, c:c + 1], in_=mx8[0:gp, 0:1])
                        mi8 = small.tile([L, 8], u32, tag="mi8s")
                        nc.vector.max_index(out=mi8[0:gp, :], in_max=mx8[0:gp, :],
                                            in_values=lt[0:gp, 0:w])
                        nc.vector.tensor_copy(out=idx_all[0:gp, c:c + 1], in_=mi8[0:gp, 0:1])
                    else:
                        # chunk maxes at sub-chunk (sw-wide) granularity
                        lt3 = lt[0:gp, 0:w].rearrange("p (a b) -> p a b", b=sw)
                        nc.vector.reduce_max(
                            out=m_all[0:gp, c * SUB:(c + 1) * SUB], in_=lt3, axis=AX.X)
                return m_all, idx_all

            def tournament(g0_, gp, m_all, rev, nslots):
                """global max + LOWEST slot attaining it (argmax tie rule)."""
                gmax = small.tile([L, 1], f32, tag="gmax")
                nc.vector.reduce_max(out=gmax[0:gp, :], in_=m_all[0:gp, :], axis=AX.X)
                sel = small.tile([L, nslots], f32, tag="sel" + str(nslots))
                nc.vector.scalar_tensor_tensor(
                    out=sel[0:gp, :], in0=m_all[0:gp, :], scalar=gmax[0:gp, 0:1],
                    in1=rev[0:gp, :], op0=Op.is_ge, op1=Op.mult)
                cmax = small.tile([L, 1], f32, tag="cmax")
                nc.vector.reduce_max(out=cmax[0:gp, :], in_=sel[0:gp, :], axis=AX.X)
                cstar = small.tile([L, 1], f32, tag="cstar")
                nc.vector.tensor_scalar(cstar[0:gp, :], cmax[0:gp, :], -1.0, float(nslots),
                                        op0=Op.mult, op1=Op.add)
                return gmax, cstar

            def finish_group(g0_, gp, hot_f, split=False):
                """hot -> W_pre gather -> tok_out rows.  split pipelines the
                gather/store pair in two row halves (shorter terminal chain)."""
                hot_i = small.tile([L, 1], i32, tag="hot_i")
                nc.vector.tensor_copy(out=hot_i[0:gp, :], in_=hot_f[0:gp, :])
                if gp == 1:
                    # single-partition indirect DMAs are rejected; pad with a
                    # zeroed second index and gather one junk row (never read)
                    nc.vector.memset(hot_i[1:2, :], 0)
                halves = [(0, gp)] if not split or gp < 32 else [
                    (0, gp // 2), (gp // 2, gp)]
                tokemb = emb.tile([L, E], f32, tag="tokemb")
                for (a, b) in halves:
                    bi = 2 if (b - a == 1 and a == 0) else b
                    nc.gpsimd.indirect_dma_start(
                        out=tokemb[a:bi, :], out_offset=None, in_=wpre_h[:],
                        in_offset=IndirectOffsetOnAxis(ap=hot_i[a:bi, 0:1], axis=0))
                steng = nc.gpsimd if d.get("STORE_SWDGE", False) else nc.sync
                for (a, b) in halves:
                    steng.dma_start(out=tokout_h[g0_ + a:g0_ + b, :], in_=tokemb[a:b, :])

            def tail_rescan_start(g0_, gp, m_all):
                """Tournament (all data ready -> no DVE stalls) + issue the
                winning-sub-chunk re-read. The dependent max_index is deferred
                (tail_rescan_end) so its DMA wait never blocks the in-order
                DVE queue while the next group streams."""
                gmax, sstar = tournament(g0_, gp, m_all, frev, nfine)
                # fine-sliced row index for the re-read: (row + g0)*nfine + sstar
                ridx_f = small.tile([L, 1], f32, tag="ridx_f")
                nc.vector.tensor_tensor(out=ridx_f[0:gp, :], in0=rowb[g0_][0:gp, :],
                                        in1=sstar[0:gp, :], op=Op.add)
                ridx_i = small.tile([L, 1], i32, tag="ridx_i")
                nc.vector.tensor_copy(out=ridx_i[0:gp, :], in_=ridx_f[0:gp, :])
                gmax8 = small.tile([L, 8], f32, tag="gmax8")
                nc.vector.tensor_copy(out=gmax8[0:gp, :],
                                      in_=gmax[0:gp, 0:1].to_broadcast([gp, 8]))

                # single-partition indirect DMAs are rejected; pad the index
                # tile with a zeroed row and gather one junk row (never read)
                gpi = 2 if gp == 1 else gp
                if gpi != gp:
                    nc.vector.memset(ridx_i[gp:gpi, :], 0)

                # re-read ONLY the winning sub-chunk (logits + CCE-add gumbel);
                # bit-matches the stream pass, so gmax is found exactly
                rs = emb.tile([L, sw], f32, tag="rs")
                nc.gpsimd.indirect_dma_start(
                    out=rs[0:gpi, :], out_offset=None, in_=lg_v,
                    in_offset=IndirectOffsetOnAxis(ap=ridx_i[0:gpi, 0:1], axis=0))
                nc.gpsimd.indirect_dma_start(
                    out=rs[0:gpi, :], out_offset=None, in_=gm_v,
                    in_offset=IndirectOffsetOnAxis(ap=ridx_i[0:gpi, 0:1], axis=0),
                    compute_op=Op.add)
                return sstar, gmax8, rs

            def tail_rescan_end(g0_, gp, state):
                sstar, gmax8, rs = state
                mi8 = small.tile([L, 8], u32, tag="mi8")
                nc.vector.max_index(out=mi8[0:gp, :], in_max=gmax8[0:gp, :],
                                    in_values=rs[0:gp, :])
                mi_f = small.tile([L, 1], f32, tag="mi_f")
                nc.vector.tensor_copy(out=mi_f[0:gp, :], in_=mi8[0:gp, 0:1])
                hot_f = small.tile([L, 1], f32, tag="hot_f")
                nc.vector.scalar_tensor_tensor(
                    out=hot_f[0:gp, :], in0=sstar[0:gp, :], scalar=float(sw),
                    in1=mi_f[0:gp, :], op0=Op.mult, op1=Op.add)
                finish_group(g0_, gp, hot_f)

            def tail_maxidx(g0_, gp, m_all, idx_all):
                _gmax, cstar = tournament(g0_, gp, m_all, srev, nsp)
                # winning span's within-span index and base offset
                junk = small.tile([L, nsp], f32, tag="junk")
                nc.vector.scalar_tensor_tensor(
                    out=junk[0:gp, :], in0=iotas[0:gp, :], scalar=cstar[0:gp, 0:1],
                    in1=idx_all[0:gp, :], op0=Op.is_equal, op1=Op.mult)
                mi_f = small.tile([L, 1], f32, tag="mi_fx")
                nc.vector.reduce_max(out=mi_f[0:gp, :], in_=junk[0:gp, :], axis=AX.X)
                junkb = small.tile([L, nsp], f32, tag="junkb")
                nc.vector.scalar_tensor_tensor(
                    out=junkb[0:gp, :], in0=iotas[0:gp, :], scalar=cstar[0:gp, 0:1],
                    in1=bases[0:gp, :], op0=Op.is_equal, op1=Op.mult)
                base_f = small.tile([L, 1], f32, tag="base_f")
                nc.vector.reduce_max(out=base_f[0:gp, :], in_=junkb[0:gp, :], axis=AX.X)
                hot_f = small.tile([L, 1], f32, tag="hot_f")
                nc.vector.tensor_tensor(out=hot_f[0:gp, :], in0=base_f[0:gp, :],
                                        in1=mi_f[0:gp, :], op=Op.add)
                finish_group(g0_, gp, hot_f, split=d.get("WSPLIT", False))

            # psg stage 1a first: only the mask load + cold-PE matmul issue,
            # so the DVE queue stays clear for the first chunk maxes
            for t in range(2):
                psg_a(t)
            pending = {d.get("ILA", 4): [lambda: [psg_a2(t) for t in range(2)]]}
            for gi, (g0_, gp, mode) in enumerate(groups):
                last = gi == len(groups) - 1
                m_all, idx_all = stream_group(g0_, gp, mode,
                                              interleave=pending)
                pending = {}
                if not last:
                    # non-terminal group: tournament + issue the re-read now;
                    # the dependent finish and psg stages 2/3 slot a few
                    # chunks into the next group's stream (data ready by then)
                    state = tail_rescan_start(g0_, gp, m_all)
                    for t in range(2):
                        psg_b(t)
                    pending = {
                        d.get("ILB", 13): [
                            lambda g0=g0_, gp_=gp, st=state: tail_rescan_end(g0, gp_, st)],
                        d.get("ILC", 8): [lambda: [psg_c(t) for t in range(2)]],
                    }
                else:
                    if len(groups) == 1:
                        for t in range(2):
                            psg_b(t)
                    if mode == "maxidx":
                        tail_maxidx(g0_, gp, m_all, idx_all)
                    else:
                        tail_rescan_end(g0_, gp, tail_rescan_start(g0_, gp, m_all))
                    if len(groups) == 1:
                        for t in range(2):
                            psg_c(t)

    return nc


_BUILD_CACHE = {}


def _get_module_v2(cap, dims_key=None, dims=None):
    key = ("v2", cap, dims_key)
    if key not in _BUILD_CACHE:
        import concourse.bacc as bacc

        nc = bacc.Bacc("TRN2", target_bir_lowering=False, debug=False)
        _build_v2(nc, cap, dims)
        nc.compile()
        _BUILD_CACHE[key] = nc
    return _BUILD_CACHE[key]


_ROWMAP_CACHE = {}


def _nearest_maps():
    """Replicate the reference's f32 grid_sample-nearest index maps with jnp
    on the same backend the reference runs on (bit-exact by construction)."""
    if "maps" not in _ROWMAP_CACHE:
        import jax.numpy as jnp

        def nearest(size):
            lin = jnp.linspace(-1.0, 1.0, size)
            ix = ((lin + 1.0) * size - 1.0) / 2.0
            return np.asarray(jnp.clip(jnp.round(ix), 0, size - 1).astype(jnp.int32))

        _ROWMAP_CACHE["maps"] = (nearest(V), nearest(E))
    return _ROWMAP_CACHE["maps"]


_TRI = None

# test/dev hooks: set TRACE=True before calling kernel() to capture an NTFF
# profile; the BassKernelResults of the last run is stored in LAST_RESULT.
TRACE = False
LAST_RESULT = None
LAST_MODULE = None


def kernel(logits, rwrt_attention_mask, psg_input_ids, word_embeddings, gumbel_noise):
    from concourse.bass_utils import run_bass_kernel_spmd

    global _TRI, LAST_RESULT, LAST_MODULE
    logits = np.ascontiguousarray(np.asarray(logits, dtype=np.float32))
    gumbel = np.ascontiguousarray(np.asarray(gumbel_noise, dtype=np.float32))
    mask = np.ascontiguousarray(np.asarray(rwrt_attention_mask, dtype=np.int32))
    psg = np.ascontiguousarray(np.asarray(psg_input_ids, dtype=np.int32))
    wte = np.ascontiguousarray(np.asarray(word_embeddings, dtype=np.float32))

    rowmap, colmap = _nearest_maps()
    # token branch reads W[rowmap][:, colmap]; constant-fold both static maps
    wpre = np.ascontiguousarray(wte[rowmap][:, colmap])
    if _TRI is None:
        _TRI = np.ascontiguousarray(np.triu(np.ones((L, L), dtype=np.float32)))

    # ---- sharding: compact the active (mask==1) tokens across cores ----
    flat_mask = mask.reshape(B * L) != 0
    ids = np.nonzero(flat_mask)[0].astype(np.int64)    # batch-major order
    n_act = int(ids.size)
    cap = max(8, -(-n_act // N_CORES))                 # tokens per core
    cap += cap & 1     # even row counts only: an odd-cap module (odd-row
                       # groups / odd-row tok_out) hit NRT_EXEC_UNIT_
                       # UNRECOVERABLE on device; one pad slot is free
    pad_id = int(ids[-1]) if n_act else 0
    ids_p = np.full(N_CORES * cap, pad_id, dtype=np.int64)
    ids_p[:n_act] = ids
    lg_flat = logits.reshape(B * L, V)
    gm_flat = gumbel.reshape(B * L, V)

    nc = _get_module_v2(cap)
    LAST_MODULE = nc

    in_maps = []
    for m in range(N_CORES):
        sl = ids_p[m * cap:(m + 1) * cap]
        im = {
            "lg": np.ascontiguousarray(lg_flat[sl]),
            "gm": np.ascontiguousarray(gm_flat[sl]),
            "mask": mask[2 * m:2 * m + 2].reshape(2 * L, 1),
            "psg": psg[2 * m:2 * m + 2].reshape(2 * L, 1),
            "wte": wte,
            "wpre": wpre,
            "tri": _TRI,
        }
        in_maps.append(im)

    try:
        LAST_RESULT = run_bass_kernel_spmd(nc, in_maps, list(range(N_CORES)), trace=TRACE)
    except Exception:
        # the axon-relayed device occasionally reports a transient
        # NRT_EXEC_UNIT_UNRECOVERABLE; re-runs (with backoff) recover it
        import time as _time

        for backoff in (2.0, 15.0):
            _time.sleep(backoff)
            try:
                LAST_RESULT = run_bass_kernel_spmd(
                    nc, in_maps, list(range(N_CORES)), trace=TRACE)
                break
            except Exception:
                if backoff == 15.0:
                    raise
    res = LAST_RESULT.results

    # ---- unshard: psg partial everywhere + scatter-add the token rows ----
    out = np.concatenate(
        [res[m]["psg_out"].reshape(2, L, E) for m in range(N_CORES)], axis=0)
    if n_act:
        tok = np.concatenate(
            [res[m]["tok_out"].reshape(cap, E) for m in range(N_CORES)], axis=0)
        flat = out.reshape(B * L, E)
        flat[ids] += tok[:n_act]
    return out


# revision 37
# speedup vs baseline: 1.0043x; 1.0043x over previous
"""Trainium2 Bass kernel: gumbel-softmax-argmax embedding lookup (end-to-end).

Reference math (nn_End2End_49495203119139):
    hot  = argmax_V(softmax((logits + gumbel)/tau))       == argmax_V(logits+gumbel)
    row  = grid_sample-nearest index map of hot            == ROWMAP[hot]  (LUT)
    tok_emb = W[row][:, col_map]   with col_map == arange(E)  (verified at runtime)
    inputs_embeds = tok_emb * mask
    psg_roll = roll(psg_ids, 1, axis=1); psg_roll[:,0] = 1
    extr  = (1 - mask[:, ::-1]) * psg_roll
    trunc = rotate_right(extr, shifts) with shifts = mask.sum(-1)   (per row)
    flag  = cumsum(trunc != 0, -1) > 0
    out   = inputs_embeds + where(flag, W[trunc], 0)

Key observation: inputs_embeds is multiplied by the attention mask, so the
argmax over the 32128-wide vocab -- the part that forces streaming the
526 MB of logits+gumbel -- is only needed for tokens with mask == 1.
The mask is an input, so sharding can compact: the host flattens the
active (b, l) positions (N of B*L), pads to C = ceil(N/8) per core, and
ships each core a compacted [C, V] slice of logits and gumbel.  Each core
streams only its C rows (vs B*L/8 = 256 for dense data-parallel).  The
passage branch (cheap) stays on the owner core of each batch row; the two
partial outputs (psg_out for all positions, tok_out rows for the active
tokens) are combined by the host during the unshard step -- an exact f32
add of the same values the dense kernel would have added on-device.

Per-core device plan (memory-bound part = streaming the compacted
logits+gumbel, 2*C*V*4 bytes ~= 45.8 MB for C=178, ~127 us HBM floor at
360 GB/s per core).  Rows go in two partition groups, (C-128, 128); per
group and per vocab span: HWDGE-load the logits span (alternating SP/ACT
DMA queues), add the gumbel span with one SWDGE inline-accumulate DMA
(CCE add; descriptors must stay <= 2048 elements), then find the span
max on DVE.  The argmax strategy differs per group because DVE is the
scarce engine after compaction (max+max_index of every span would put
~145 us on DVE vs the ~135 us DMA floor):
  - FIRST group (C-128 rows): max-only streaming -- one reduce_max per
    span at 251-element sub-span granularity.  A short tournament picks
    the LOWEST sub-span attaining the global max (argmax tie rule), then
    that sub-span alone is re-read (indirect row gather + CCE-add) and
    ONE max_index against the broadcast global max finds the in-span
    position -- exact because the CCE re-add bit-matches the stream
    pass.  Its tail hides under the second group's stream; the max_index
    is deferred a few spans into it so the re-read's DMA latency never
    blocks the in-order DVE queue.
  - LAST group (128 rows; carries the terminal tail): per-span max +
    max_index inline (affordable: its 5.7 us/span DMA outpaces the
    4.4 us/span DVE cost), so after the final span only the tournament
    and the embedding gather remain.  The vocab end is tapered into
    shrinking spans so the final span's DVE work is short.
  - hot -> token embedding via ONE indirect gather from a host-built
    table W_pre = W[rowmap][:, colmap] (constant-folds the grid_sample
    LUTs, removing a dependent gather from the critical tail).
  - the passage branch is pure index arithmetic on [128,1] tiles: the
    reverse/roll/rotate are folded into gather indices modulo L, the
    mask-sum and cumsum are 0/1 matmuls against ones/triangular matrices
    (exact in any PE precision).  psg_out = flag * W[trunc] is stored for
    every position.  It is emitted in three stages (index math + row
    gathers up front, combine after the first group's stream, flag+store
    mid second stream) so its cross-engine waits never stall the DVE
    queue ahead of stream work.
"""

import numpy as np

B = 16
L = 128
V = 32128
E = 768
N_CORES = 8
CH = 2008                     # vocab chunk (free dim) per streamed tile;
                              # <= 2048 elements so a gumbel chunk is ONE
                              # CCE-add DMA descriptor per partition row
NCH = V // CH                 # 16 chunks


def _build_v2(nc_mod, cap, dims=None):
    """Per-core module for the compacted layout. cap = tokens per core."""
    import concourse.tile as tile
    from concourse import bass, mybir
    from concourse.bass import IndirectOffsetOnAxis

    d = dims or {}
    ch = d.get("CH", CH)
    nch = V // ch
    lbufs = d.get("LBUFS", 10)
    # rescan-mode groups track chunk maxes at sub-chunk granularity so the
    # winning-chunk re-read (and its max_index) touch only SW elements
    SUB = ch // 251
    sw = 251                      # V == nch*SUB*sw exactly
    nfine = nch * SUB             # 128 fine slots
    # maxidx-mode groups taper the end of the vocab so the last spans'
    # (max + max_index) DVE work after the final CCE-add is short
    taper = d.get("TAPER", (1255, 1255, 1255, 251) if ch == 2008 else
                  (502, 502, 502, 502))
    nfull = nch - sum(taper) // ch
    assert sum(taper) % ch == 0
    spans_mi = [(c * ch, ch) for c in range(nfull)]
    tail_lo = nfull * ch
    for w_ in taper:
        spans_mi.append((tail_lo, w_))
        tail_lo += w_
    assert tail_lo == V
    nsp = len(spans_mi)

    nc = nc_mod
    f32 = mybir.dt.float32
    i32 = mybir.dt.int32
    u32 = mybir.dt.uint32
    Op = mybir.AluOpType
    AX = mybir.AxisListType

    # partition groups covering the cap rows.  The LAST group carries the
    # terminal tail, so it uses baseline per-chunk max_index (short tail, no
    # re-read) -- affordable only when its DMA per chunk (rows*ch*8B/360GBps)
    # outpaces the ~4.4us/chunk of DVE work, i.e. rows >= ~100.  Small groups
    # use max-only streaming with a winning-chunk re-read; their longer tail
    # hides under the next group's stream.
    groups = []  # (row0, rows, mode)
    g1mode = d.get("G1MODE", "maxidx")
    if cap > 128:
        g1rows = min(d.get("G1ROWS", 128), cap)
        g0rows = cap - g1rows
        if g0rows < 48:
            # very small streaming groups crashed the device (validated down
            # to 48 rows); rebalance and drop to rescan mode if the terminal
            # group gets too small for inline max_index to keep DVE fed
            g0rows = 48
            g1rows = cap - g0rows
        mode1 = g1mode if g1rows >= 100 else "rescan"
        groups.append((0, g0rows, "rescan"))
        groups.append((g0rows, g1rows, mode1))
    else:
        groups.append((0, cap, g1mode if cap >= 128 else "rescan"))

    lg_h = nc.dram_tensor("lg", [cap, V], f32, kind="ExternalInput")
    gm_h = nc.dram_tensor("gm", [cap, V], f32, kind="ExternalInput")
    mask_h = nc.dram_tensor("mask", [2 * L, 1], i32, kind="ExternalInput")
    psg_h = nc.dram_tensor("psg", [2 * L, 1], i32, kind="ExternalInput")
    wte_h = nc.dram_tensor("wte", [V, E], f32, kind="ExternalInput")
    wpre_h = nc.dram_tensor("wpre", [V, E], f32, kind="ExternalInput")
    tri_h = nc.dram_tensor("tri", [L, L], f32, kind="ExternalInput")
    psgout_h = nc.dram_tensor("psg_out", [2 * L, E], f32, kind="ExternalOutput")
    tokout_h = nc.dram_tensor("tok_out", [cap, E], f32, kind="ExternalOutput")

    # fine-sliced row views for the data-dependent winning-slot re-read
    lg_v = lg_h[:].rearrange("r (c w) -> (r c) w", w=sw)
    gm_v = gm_h[:].rearrange("r (c w) -> (r c) w", w=sw)

    with tile.TileContext(nc) as tc:
        with (
            tc.tile_pool(name="lpool", bufs=lbufs) as lpool,
            tc.tile_pool(name="lpool1", bufs=d.get("LBUFS1", 5)) as lpool1,
            tc.tile_pool(name="stats", bufs=2) as stats,
            tc.tile_pool(name="small", bufs=2) as small,
            tc.tile_pool(name="emb", bufs=2) as emb,
            tc.tile_pool(name="consts", bufs=1) as consts,
            tc.tile_pool(name="psum", bufs=2, space="PSUM") as psum,
        ):
            # ---- per-core constants (built once) ----
            ones_mat = consts.tile([L, L], f32)
            nc.vector.memset(ones_mat[:], 1.0)
            iota_p_i = consts.tile([L, 1], i32)
            nc.gpsimd.iota(iota_p_i[:], pattern=[[1, 1]], base=0, channel_multiplier=1)
            iota_p = consts.tile([L, 1], f32)
            nc.vector.tensor_copy(out=iota_p[:], in_=iota_p_i[:])

            # tri[i,j] = (j >= i) built on device: one less DMA
            tri_sb = consts.tile([L, L], f32)
            if d.get("TRI_DEV", True):
                trij_i = consts.tile([L, L], i32)
                nc.gpsimd.iota(trij_i[:], pattern=[[1, L]], base=0, channel_multiplier=0)
                trij = consts.tile([L, L], f32)
                nc.vector.tensor_copy(out=trij[:], in_=trij_i[:])
                nc.vector.tensor_scalar(tri_sb[:], trij[:], iota_p[:, 0:1], None, op0=Op.is_ge)
            else:
                nc.gpsimd.dma_start(out=tri_sb[:], in_=tri_h[:])

            need_mi = any(m == "maxidx" for (_, _, m) in groups)
            need_rs = any(m == "rescan" for (_, _, m) in groups)
            if need_mi:
                iotas_i = consts.tile([L, nsp], i32)
                nc.gpsimd.iota(iotas_i[:], pattern=[[1, nsp]], base=0, channel_multiplier=0)
                iotas = consts.tile([L, nsp], f32)
                nc.vector.tensor_copy(out=iotas[:], in_=iotas_i[:])
                # srev[c] = nsp - c (used to pick the LOWEST span attaining the max)
                srev = consts.tile([L, nsp], f32)
                nc.vector.tensor_scalar(srev[:], iotas[:], -1.0, float(nsp), op0=Op.mult, op1=Op.add)
                # per-span start offsets (hot = bases[c*] + within-span index)
                bases = consts.tile([L, nsp], f32)
                nc.vector.tensor_scalar(bases[:], iotas[:], float(ch), None, op0=Op.mult)
                for ci, (lo_c, _w) in enumerate(spans_mi):
                    if lo_c != ci * ch:
                        nc.vector.memset(bases[:, ci:ci + 1], float(lo_c))
            if need_rs:
                iotaf_i = consts.tile([L, nfine], i32)
                nc.gpsimd.iota(iotaf_i[:], pattern=[[1, nfine]], base=0, channel_multiplier=0)
                iotaf = consts.tile([L, nfine], f32)
                nc.vector.tensor_copy(out=iotaf[:], in_=iotaf_i[:])
                frev = consts.tile([L, nfine], f32)
                nc.vector.tensor_scalar(frev[:], iotaf[:], -1.0, float(nfine), op0=Op.mult, op1=Op.add)
            # per-partition fine-sliced row base: (row + group offset) * nfine
            rowb = {}
            for (g0_, _gp, mode) in groups:
                if mode != "rescan":
                    continue
                rb = consts.tile([L, 1], f32)
                nc.vector.tensor_scalar(rb[:], iota_p[:], float(nfine), float(g0_ * nfine),
                                        op0=Op.mult, op1=Op.add)
                rowb[g0_] = rb

            psg_state = {}

            def psg_a(t):
                """psg stage 1: mask load, length, gather indices; issues the
                mask/psg row gathers.  All DVE ops here depend only on the
                mask load + ones matmul, so they run during kernel startup."""
                tok = slice(t * L, (t + 1) * L)
                mask_i = small.tile([L, 1], i32, tag="mask_i")
                nc.scalar.dma_start(out=mask_i[:], in_=mask_h[tok, :])
                mask_f = small.tile([L, 1], f32, tag="mask_f")
                nc.vector.tensor_copy(out=mask_f[:], in_=mask_i[:])

                # s (broadcast to all partitions) = sum(mask) via ones matmul
                s_ps = psum.tile([L, 1], f32, tag="s_ps")
                nc.tensor.matmul(out=s_ps[:], lhsT=ones_mat[:], rhs=mask_f[:], start=True, stop=True)
                psg_state[t] = dict(s_ps=s_ps)

            def psg_a2(t):
                """psg stage 1b: index arithmetic (the cold-PE matmul from
                stage 1a is long done, so nothing here stalls the DVE queue);
                issues the mask/psg row gathers."""
                st = psg_state[t]
                s_ps = st["s_ps"]
                s_bc = small.tile([L, 1], f32, tag="s_bc")
                nc.vector.tensor_copy(out=s_bc[:], in_=s_ps[:])

                def mod_l(x_ap, lo_fix=True, hi_fix=True, tagp=""):
                    # x <- x mod L for x in (-L, 2L)
                    if hi_fix:
                        ge = small.tile([L, 1], f32, tag="ge" + tagp)
                        nc.vector.tensor_scalar(ge[:], x_ap, float(L), None, op0=Op.is_ge)
                        nc.vector.scalar_tensor_tensor(
                            out=x_ap, in0=ge[:], scalar=-float(L), in1=x_ap, op0=Op.mult, op1=Op.add)
                    if lo_fix:
                        lt_ = small.tile([L, 1], f32, tag="lt" + tagp)
                        nc.vector.tensor_scalar(lt_[:], x_ap, 0.0, None, op0=Op.is_lt)
                        nc.vector.scalar_tensor_tensor(
                            out=x_ap, in0=lt_[:], scalar=float(L), in1=x_ap, op0=Op.mult, op1=Op.add)

                # fidx = (L-1 + s - l) mod L   (flipped-mask gather index)
                fidx = small.tile([L, 1], f32, tag="fidx")
                nc.vector.scalar_tensor_tensor(
                    out=fidx[:], in0=s_bc[:], scalar=float(L - 1), in1=iota_p[:],
                    op0=Op.add, op1=Op.subtract)
                mod_l(fidx[:], lo_fix=False, tagp="f")

                # pidx = (L-1 - s + l) mod L   (rolled-psg gather index)
                pidx = small.tile([L, 1], f32, tag="pidx")
                nc.vector.scalar_tensor_tensor(
                    out=pidx[:], in0=s_bc[:], scalar=-1.0, in1=iota_p[:],
                    op0=Op.mult, op1=Op.add)
                nc.vector.tensor_scalar(pidx[:], pidx[:], float(L - 1), None, op0=Op.add)
                mod_l(pidx[:], tagp="p")

                # k = (l - s) mod L ; BOS position is k == 0
                kk = small.tile([L, 1], f32, tag="kk")
                nc.vector.scalar_tensor_tensor(
                    out=kk[:], in0=s_bc[:], scalar=-1.0, in1=iota_p[:],
                    op0=Op.mult, op1=Op.add)
                mod_l(kk[:], hi_fix=False, tagp="k")
                bos = small.tile([L, 1], f32, tag="bos")
                nc.vector.tensor_scalar(bos[:], kk[:], 0.0, None, op0=Op.is_equal)

                # gather mask[fidx] and psg[pidx] (within this batch row)
                fr_i = small.tile([L, 1], i32, tag="fr_i")
                nc.vector.tensor_scalar(fidx[:], fidx[:], float(t * L), None, op0=Op.add)
                nc.vector.tensor_copy(out=fr_i[:], in_=fidx[:])
                mrev = small.tile([L, 1], i32, tag="mrev")
                nc.gpsimd.indirect_dma_start(
                    out=mrev[:], out_offset=None, in_=mask_h[:],
                    in_offset=IndirectOffsetOnAxis(ap=fr_i[:, 0:1], axis=0),
                )
                pr_i = small.tile([L, 1], i32, tag="pr_i")
                nc.vector.tensor_scalar(pidx[:], pidx[:], float(t * L), None, op0=Op.add)
                nc.vector.tensor_copy(out=pr_i[:], in_=pidx[:])
                prot = small.tile([L, 1], i32, tag="prot")
                nc.gpsimd.indirect_dma_start(
                    out=prot[:], out_offset=None, in_=psg_h[:],
                    in_offset=IndirectOffsetOnAxis(ap=pr_i[:, 0:1], axis=0),
                )
                st.update(bos=bos, mrev=mrev, prot=prot)

            def psg_b(t):
                """psg stage 2: combine the gathered rows (long since arrived)
                into trunc/flag; issues the psg embedding gather."""
                st = psg_state[t]
                bos, mrev, prot = st["bos"], st["mrev"], st["prot"]
                # f_rot = 1 - mask[fidx]
                mrev_f = small.tile([L, 1], f32, tag="mrev_f")
                nc.vector.tensor_copy(out=mrev_f[:], in_=mrev[:])
                frot = small.tile([L, 1], f32, tag="frot")
                nc.vector.tensor_scalar(frot[:], mrev_f[:], -1.0, 1.0, op0=Op.mult, op1=Op.add)
                # psg_rot = bos ? 1 : psg[pidx]
                prot_f = small.tile([L, 1], f32, tag="prot_f")
                nc.vector.tensor_copy(out=prot_f[:], in_=prot[:])
                nbos = small.tile([L, 1], f32, tag="nbos")
                nc.vector.tensor_scalar(nbos[:], bos[:], -1.0, 1.0, op0=Op.mult, op1=Op.add)
                nc.vector.tensor_tensor(out=prot_f[:], in0=prot_f[:], in1=nbos[:], op=Op.mult)
                nc.vector.tensor_tensor(out=prot_f[:], in0=prot_f[:], in1=bos[:], op=Op.add)
                # trunc = f_rot * psg_rot
                trunc = small.tile([L, 1], f32, tag="trunc")
                nc.vector.tensor_tensor(out=trunc[:], in0=frot[:], in1=prot_f[:], op=Op.mult)

                # flag = cumsum(trunc != 0) > 0 via triangular matmul
                nz = small.tile([L, 1], f32, tag="nz")
                nc.vector.tensor_scalar(nz[:], trunc[:], 0.0, None, op0=Op.not_equal)
                cnt_ps = psum.tile([L, 1], f32, tag="cnt_ps")
                nc.tensor.matmul(out=cnt_ps[:], lhsT=tri_sb[:], rhs=nz[:], start=True, stop=True)

                trunc_i = small.tile([L, 1], i32, tag="trunc_i")
                nc.vector.tensor_copy(out=trunc_i[:], in_=trunc[:])
                psgemb = emb.tile([L, E], f32, tag="psgemb")
                nc.gpsimd.indirect_dma_start(
                    out=psgemb[:], out_offset=None, in_=wte_h[:],
                    in_offset=IndirectOffsetOnAxis(ap=trunc_i[:, 0:1], axis=0),
                )
                st["cnt_ps"] = cnt_ps
                st["psgemb"] = psgemb

            def psg_c(t):
                """psg stage 3: flag-mask the embeddings and store."""
                st = psg_state[t]
                tok = slice(t * L, (t + 1) * L)
                flag = small.tile([L, 1], f32, tag="flag")
                nc.vector.tensor_scalar(flag[:], st["cnt_ps"][:], 0.0, None, op0=Op.is_gt)
                outp = emb.tile([L, E], f32, tag="outp")
                nc.vector.tensor_scalar(outp[:], st["psgemb"][:], flag[:, 0:1],
                                        None, op0=Op.mult)
                nc.sync.dma_start(out=psgout_h[tok, :], in_=outp[:])

            def stream_group(g0_, gp, mode, interleave=None):
                """DMA-bound pass over the vocab for rows [g0_, g0_+gp):
                per span, load logits, accumulate gumbel in the DMA
                datapath, then track span maxes (and, in maxidx mode, the
                first within-span argmax position).  `interleave` is the
                prior group's deferred rescan finish, emitted a few chunks
                in -- by then its re-read data has arrived, so the max_index
                never stalls the in-order DVE queue."""
                rows = slice(g0_, g0_ + gp)
                idx_all = None
                if mode == "maxidx":
                    m_all = stats.tile([L, nsp], f32, tag="m_all" + mode)
                    idx_all = stats.tile([L, nsp], f32, tag="idx_all")
                    spans = spans_mi
                else:
                    m_all = stats.tile([L, nfine], f32, tag="m_all" + mode)
                    spans = [(c * ch, ch) for c in range(nch)]
                pool = lpool if g0_ == 0 else lpool1
                for c, (lo, w) in enumerate(spans):
                    for fn in (interleave or {}).get(c, []):
                        fn()
                    lt = pool.tile([L, ch], f32, tag="lt")
                    ldeng = nc.scalar if c % 2 else nc.sync
                    ldeng.dma_start(out=lt[0:gp, 0:w], in_=lg_h[rows, lo:lo + w])
                    nc.gpsimd.dma_start(
                        out=lt[0:gp, 0:w], in_=gm_h[rows, lo:lo + w], accum_op=Op.add)
                    if mode == "maxidx":
                        mx8 = small.tile([L, 8], f32, tag="mx8")
                        nc.vector.max(out=mx8[0:gp, :], in_=lt[0:gp, 0:w])
                        nc.vector.tensor_copy(out=m_all[0:gp, c:c + 1], in_=mx8[0:gp, 0:1])
                        mi8 = small.tile([L, 8], u32, tag="mi8s")
                        nc.vector.max_index(out=mi8[0:gp, :], in_max=mx8[0:gp, :],
                                            in_values=lt[0:gp, 0:w])
                        nc.vector.tensor_copy(out=idx_all[0:gp, c:c + 1], in_=mi8[0:gp, 0:1])
                    else:
                        # chunk maxes at sub-chunk (sw-wide) granularity
                        lt3 = lt[0:gp, 0:w].rearrange("p (a b) -> p a b", b=sw)
                        nc.vector.reduce_max(
                            out=m_all[0:gp, c * SUB:(c + 1) * SUB], in_=lt3, axis=AX.X)
                return m_all, idx_all

            def tournament(g0_, gp, m_all, rev, nslots):
                """global max + LOWEST slot attaining it (argmax tie rule)."""
                gmax = small.tile([L, 1], f32, tag="gmax")
                nc.vector.reduce_max(out=gmax[0:gp, :], in_=m_all[0:gp, :], axis=AX.X)
                sel = small.tile([L, nslots], f32, tag="sel" + str(nslots))
                nc.vector.scalar_tensor_tensor(
                    out=sel[0:gp, :], in0=m_all[0:gp, :], scalar=gmax[0:gp, 0:1],
                    in1=rev[0:gp, :], op0=Op.is_ge, op1=Op.mult)
                cmax = small.tile([L, 1], f32, tag="cmax")
                nc.vector.reduce_max(out=cmax[0:gp, :], in_=sel[0:gp, :], axis=AX.X)
                cstar = small.tile([L, 1], f32, tag="cstar")
                nc.vector.tensor_scalar(cstar[0:gp, :], cmax[0:gp, :], -1.0, float(nslots),
                                        op0=Op.mult, op1=Op.add)
                return gmax, cstar

            def finish_group(g0_, gp, hot_f, split=False):
                """hot -> W_pre gather -> tok_out rows.  split pipelines the
                gather/store pair in two row halves (shorter terminal chain)."""
                hot_i = small.tile([L, 1], i32, tag="hot_i")
                nc.vector.tensor_copy(out=hot_i[0:gp, :], in_=hot_f[0:gp, :])
                if gp == 1:
                    # single-partition indirect DMAs are rejected; pad with a
                    # zeroed second index and gather one junk row (never read)
                    nc.vector.memset(hot_i[1:2, :], 0)
                halves = [(0, gp)] if not split or gp < 32 else [
                    (0, gp // 2), (gp // 2, gp)]
                tokemb = emb.tile([L, E], f32, tag="tokemb")
                for (a, b) in halves:
                    bi = 2 if (b - a == 1 and a == 0) else b
                    nc.gpsimd.indirect_dma_start(
                        out=tokemb[a:bi, :], out_offset=None, in_=wpre_h[:],
                        in_offset=IndirectOffsetOnAxis(ap=hot_i[a:bi, 0:1], axis=0))
                steng = {"swdge": nc.gpsimd, "act": nc.scalar}.get(
                    d.get("STORE_ENG", "sp"), nc.sync)
                for (a, b) in halves:
                    steng.dma_start(out=tokout_h[g0_ + a:g0_ + b, :], in_=tokemb[a:b, :])

            def tail_rescan_start(g0_, gp, m_all):
                """Tournament (all data ready -> no DVE stalls) + issue the
                winning-sub-chunk re-read. The dependent max_index is deferred
                (tail_rescan_end) so its DMA wait never blocks the in-order
                DVE queue while the next group streams."""
                gmax, sstar = tournament(g0_, gp, m_all, frev, nfine)
                # fine-sliced row index for the re-read: (row + g0)*nfine + sstar
                ridx_f = small.tile([L, 1], f32, tag="ridx_f")
                nc.vector.tensor_tensor(out=ridx_f[0:gp, :], in0=rowb[g0_][0:gp, :],
                                        in1=sstar[0:gp, :], op=Op.add)
                ridx_i = small.tile([L, 1], i32, tag="ridx_i")
                nc.vector.tensor_copy(out=ridx_i[0:gp, :], in_=ridx_f[0:gp, :])
                gmax8 = small.tile([L, 8], f32, tag="gmax8")
                nc.vector.tensor_copy(out=gmax8[0:gp, :],
                                      in_=gmax[0:gp, 0:1].to_broadcast([gp, 8]))

                # single-partition indirect DMAs are rejected; pad the index
                # tile with a zeroed row and gather one junk row (never read)
                gpi = 2 if gp == 1 else gp
                if gpi != gp:
                    nc.vector.memset(ridx_i[gp:gpi, :], 0)

                # re-read ONLY the winning sub-chunk (logits + CCE-add gumbel);
                # bit-matches the stream pass, so gmax is found exactly
                rs = emb.tile([L, sw], f32, tag="rs")
                nc.gpsimd.indirect_dma_start(
                    out=rs[0:gpi, :], out_offset=None, in_=lg_v,
                    in_offset=IndirectOffsetOnAxis(ap=ridx_i[0:gpi, 0:1], axis=0))
                nc.gpsimd.indirect_dma_start(
                    out=rs[0:gpi, :], out_offset=None, in_=gm_v,
                    in_offset=IndirectOffsetOnAxis(ap=ridx_i[0:gpi, 0:1], axis=0),
                    compute_op=Op.add)
                return sstar, gmax8, rs

            def tail_rescan_end(g0_, gp, state):
                sstar, gmax8, rs = state
                mi8 = small.tile([L, 8], u32, tag="mi8")
                nc.vector.max_index(out=mi8[0:gp, :], in_max=gmax8[0:gp, :],
                                    in_values=rs[0:gp, :])
                mi_f = small.tile([L, 1], f32, tag="mi_f")
                nc.vector.tensor_copy(out=mi_f[0:gp, :], in_=mi8[0:gp, 0:1])
                hot_f = small.tile([L, 1], f32, tag="hot_f")
                nc.vector.scalar_tensor_tensor(
                    out=hot_f[0:gp, :], in0=sstar[0:gp, :], scalar=float(sw),
                    in1=mi_f[0:gp, :], op0=Op.mult, op1=Op.add)
                finish_group(g0_, gp, hot_f)

            def tail_maxidx(g0_, gp, m_all, idx_all):
                _gmax, cstar = tournament(g0_, gp, m_all, srev, nsp)
                # winning span's within-span index and base offset
                junk = small.tile([L, nsp], f32, tag="junk")
                nc.vector.scalar_tensor_tensor(
                    out=junk[0:gp, :], in0=iotas[0:gp, :], scalar=cstar[0:gp, 0:1],
                    in1=idx_all[0:gp, :], op0=Op.is_equal, op1=Op.mult)
                mi_f = small.tile([L, 1], f32, tag="mi_fx")
                nc.vector.reduce_max(out=mi_f[0:gp, :], in_=junk[0:gp, :], axis=AX.X)
                junkb = small.tile([L, nsp], f32, tag="junkb")
                nc.vector.scalar_tensor_tensor(
                    out=junkb[0:gp, :], in0=iotas[0:gp, :], scalar=cstar[0:gp, 0:1],
                    in1=bases[0:gp, :], op0=Op.is_equal, op1=Op.mult)
                base_f = small.tile([L, 1], f32, tag="base_f")
                nc.vector.reduce_max(out=base_f[0:gp, :], in_=junkb[0:gp, :], axis=AX.X)
                hot_f = small.tile([L, 1], f32, tag="hot_f")
                nc.vector.tensor_tensor(out=hot_f[0:gp, :], in0=base_f[0:gp, :],
                                        in1=mi_f[0:gp, :], op=Op.add)
                finish_group(g0_, gp, hot_f, split=d.get("WSPLIT", False))

            # psg stage 1a first: only the mask load + cold-PE matmul issue,
            # so the DVE queue stays clear for the first chunk maxes
            for t in range(2):
                psg_a(t)
            pending = {d.get("ILA", 4): [lambda: [psg_a2(t) for t in range(2)]]}
            for gi, (g0_, gp, mode) in enumerate(groups):
                last = gi == len(groups) - 1
                m_all, idx_all = stream_group(g0_, gp, mode,
                                              interleave=pending)
                pending = {}
                if not last:
                    # non-terminal group: tournament + issue the re-read now;
                    # the dependent finish and psg stages 2/3 slot a few
                    # chunks into the next group's stream (data ready by then)
                    state = tail_rescan_start(g0_, gp, m_all)
                    for t in range(2):
                        psg_b(t)
                    pending = {
                        d.get("ILB", 13): [
                            lambda g0=g0_, gp_=gp, st=state: tail_rescan_end(g0, gp_, st)],
                        d.get("ILC", 8): [lambda: [psg_c(t) for t in range(2)]],
                    }
                else:
                    if len(groups) == 1:
                        for t in range(2):
                            psg_b(t)
                    if mode == "maxidx":
                        tail_maxidx(g0_, gp, m_all, idx_all)
                    else:
                        tail_rescan_end(g0_, gp, tail_rescan_start(g0_, gp, m_all))
                    if len(groups) == 1:
                        for t in range(2):
                            psg_c(t)

    return nc


_BUILD_CACHE = {}


def _get_module_v2(cap, dims_key=None, dims=None):
    key = ("v2", cap, dims_key)
    if key not in _BUILD_CACHE:
        import concourse.bacc as bacc

        nc = bacc.Bacc("TRN2", target_bir_lowering=False, debug=False)
        _build_v2(nc, cap, dims)
        nc.compile()
        _BUILD_CACHE[key] = nc
    return _BUILD_CACHE[key]


_ROWMAP_CACHE = {}


def _nearest_maps():
    """Replicate the reference's f32 grid_sample-nearest index maps with jnp
    on the same backend the reference runs on (bit-exact by construction)."""
    if "maps" not in _ROWMAP_CACHE:
        import jax.numpy as jnp

        def nearest(size):
            lin = jnp.linspace(-1.0, 1.0, size)
            ix = ((lin + 1.0) * size - 1.0) / 2.0
            return np.asarray(jnp.clip(jnp.round(ix), 0, size - 1).astype(jnp.int32))

        _ROWMAP_CACHE["maps"] = (nearest(V), nearest(E))
    return _ROWMAP_CACHE["maps"]


_TRI = None

# test/dev hooks: set TRACE=True before calling kernel() to capture an NTFF
# profile; the BassKernelResults of the last run is stored in LAST_RESULT.
TRACE = False
LAST_RESULT = None
LAST_MODULE = None


def kernel(logits, rwrt_attention_mask, psg_input_ids, word_embeddings, gumbel_noise):
    from concourse.bass_utils import run_bass_kernel_spmd

    global _TRI, LAST_RESULT, LAST_MODULE
    logits = np.ascontiguousarray(np.asarray(logits, dtype=np.float32))
    gumbel = np.ascontiguousarray(np.asarray(gumbel_noise, dtype=np.float32))
    mask = np.ascontiguousarray(np.asarray(rwrt_attention_mask, dtype=np.int32))
    psg = np.ascontiguousarray(np.asarray(psg_input_ids, dtype=np.int32))
    wte = np.ascontiguousarray(np.asarray(word_embeddings, dtype=np.float32))

    rowmap, colmap = _nearest_maps()
    # token branch reads W[rowmap][:, colmap]; constant-fold both static maps
    wpre = np.ascontiguousarray(wte[rowmap][:, colmap])
    if _TRI is None:
        _TRI = np.ascontiguousarray(np.triu(np.ones((L, L), dtype=np.float32)))

    # ---- sharding: compact the active (mask==1) tokens across cores ----
    flat_mask = mask.reshape(B * L) != 0
    ids = np.nonzero(flat_mask)[0].astype(np.int64)    # batch-major order
    n_act = int(ids.size)
    cap = max(8, -(-n_act // N_CORES))                 # tokens per core
    cap += cap & 1     # even row counts only: an odd-cap module (odd-row
                       # groups / odd-row tok_out) hit NRT_EXEC_UNIT_
                       # UNRECOVERABLE on device; one pad slot is free
    pad_id = int(ids[-1]) if n_act else 0
    ids_p = np.full(N_CORES * cap, pad_id, dtype=np.int64)
    ids_p[:n_act] = ids
    lg_flat = logits.reshape(B * L, V)
    gm_flat = gumbel.reshape(B * L, V)

    nc = _get_module_v2(cap)
    LAST_MODULE = nc

    in_maps = []
    for m in range(N_CORES):
        sl = ids_p[m * cap:(m + 1) * cap]
        im = {
            "lg": np.ascontiguousarray(lg_flat[sl]),
            "gm": np.ascontiguousarray(gm_flat[sl]),
            "mask": mask[2 * m:2 * m + 2].reshape(2 * L, 1),
            "psg": psg[2 * m:2 * m + 2].reshape(2 * L, 1),
            "wte": wte,
            "wpre": wpre,
            "tri": _TRI,
        }
        in_maps.append(im)

    try:
        LAST_RESULT = run_bass_kernel_spmd(nc, in_maps, list(range(N_CORES)), trace=TRACE)
    except Exception:
        # the axon-relayed device occasionally reports a transient
        # NRT_EXEC_UNIT_UNRECOVERABLE; re-runs (with backoff) recover it
        import time as _time

        for backoff in (2.0, 15.0):
            _time.sleep(backoff)
            try:
                LAST_RESULT = run_bass_kernel_spmd(
                    nc, in_maps, list(range(N_CORES)), trace=TRACE)
                break
            except Exception:
                if backoff == 15.0:
                    raise
    res = LAST_RESULT.results

    # ---- unshard: psg partial everywhere + scatter-add the token rows ----
    out = np.concatenate(
        [res[m]["psg_out"].reshape(2, L, E) for m in range(N_CORES)], axis=0)
    if n_act:
        tok = np.concatenate(
            [res[m]["tok_out"].reshape(cap, E) for m in range(N_CORES)], axis=0)
        flat = out.reshape(B * L, E)
        flat[ids] += tok[:n_act]
    return out


# revision 38
# speedup vs baseline: 1.0050x; 1.0008x over previous
"""Trainium2 Bass kernel: gumbel-softmax-argmax embedding lookup (end-to-end).

Reference math (nn_End2End_49495203119139):
    hot  = argmax_V(softmax((logits + gumbel)/tau))       == argmax_V(logits+gumbel)
    row  = grid_sample-nearest index map of hot            == ROWMAP[hot]  (LUT)
    tok_emb = W[row][:, col_map]   with col_map == arange(E)  (verified at runtime)
    inputs_embeds = tok_emb * mask
    psg_roll = roll(psg_ids, 1, axis=1); psg_roll[:,0] = 1
    extr  = (1 - mask[:, ::-1]) * psg_roll
    trunc = rotate_right(extr, shifts) with shifts = mask.sum(-1)   (per row)
    flag  = cumsum(trunc != 0, -1) > 0
    out   = inputs_embeds + where(flag, W[trunc], 0)

Key observation: inputs_embeds is multiplied by the attention mask, so the
argmax over the 32128-wide vocab -- the part that forces streaming the
526 MB of logits+gumbel -- is only needed for tokens with mask == 1.
The mask is an input, so sharding can compact: the host flattens the
active (b, l) positions (N of B*L), pads to C = ceil(N/8) per core, and
ships each core a compacted [C, V] slice of logits and gumbel.  Each core
streams only its C rows (vs B*L/8 = 256 for dense data-parallel).  The
passage branch (cheap) stays on the owner core of each batch row; the two
partial outputs (psg_out for all positions, tok_out rows for the active
tokens) are combined by the host during the unshard step -- an exact f32
add of the same values the dense kernel would have added on-device.

Per-core device plan (memory-bound part = streaming the compacted
logits+gumbel, 2*C*V*4 bytes ~= 45.8 MB for C=178, ~127 us HBM floor at
360 GB/s per core).  Rows go in two partition groups, (C-128, 128); per
group and per vocab span: HWDGE-load the logits span (alternating SP/ACT
DMA queues), add the gumbel span with one SWDGE inline-accumulate DMA
(CCE add; descriptors must stay <= 2048 elements), then find the span
max on DVE.  The argmax strategy differs per group because DVE is the
scarce engine after compaction (max+max_index of every span would put
~145 us on DVE vs the ~135 us DMA floor):
  - FIRST group (C-128 rows): max-only streaming -- one reduce_max per
    span at 251-element sub-span granularity.  A short tournament picks
    the LOWEST sub-span attaining the global max (argmax tie rule), then
    that sub-span alone is re-read (indirect row gather + CCE-add) and
    ONE max_index against the broadcast global max finds the in-span
    position -- exact because the CCE re-add bit-matches the stream
    pass.  Its tail hides under the second group's stream; the max_index
    is deferred a few spans into it so the re-read's DMA latency never
    blocks the in-order DVE queue.
  - LAST group (128 rows; carries the terminal tail): per-span max +
    max_index inline (affordable: its 5.7 us/span DMA outpaces the
    4.4 us/span DVE cost), so after the final span only the tournament
    and the embedding gather remain.  The vocab end is tapered into
    shrinking spans so the final span's DVE work is short.
  - hot -> token embedding via ONE indirect gather from a host-built
    table W_pre = W[rowmap][:, colmap] (constant-folds the grid_sample
    LUTs, removing a dependent gather from the critical tail).
  - the passage branch is pure index arithmetic on [128,1] tiles: the
    reverse/roll/rotate are folded into gather indices modulo L, the
    mask-sum and cumsum are 0/1 matmuls against ones/triangular matrices
    (exact in any PE precision).  psg_out = flag * W[trunc] is stored for
    every position.  It is emitted in three stages (index math + row
    gathers up front, combine after the first group's stream, flag+store
    mid second stream) so its cross-engine waits never stall the DVE
    queue ahead of stream work.
"""

import numpy as np

B = 16
L = 128
V = 32128
E = 768
N_CORES = 8
CH = 2008                     # vocab chunk (free dim) per streamed tile;
                              # <= 2048 elements so a gumbel chunk is ONE
                              # CCE-add DMA descriptor per partition row
NCH = V // CH                 # 16 chunks


def _build_v2(nc_mod, cap, dims=None):
    """Per-core module for the compacted layout. cap = tokens per core."""
    import concourse.tile as tile
    from concourse import bass, mybir
    from concourse.bass import IndirectOffsetOnAxis

    d = dims or {}
    ch = d.get("CH", CH)
    nch = V // ch
    lbufs = d.get("LBUFS", 10)
    # rescan-mode groups track chunk maxes at sub-chunk granularity so the
    # winning-chunk re-read (and its max_index) touch only SW elements
    SUB = ch // 251
    sw = 251                      # V == nch*SUB*sw exactly
    nfine = nch * SUB             # 128 fine slots
    # maxidx-mode groups taper the end of the vocab so the last spans'
    # (max + max_index) DVE work after the final CCE-add is short
    taper = d.get("TAPER", (1255, 1255, 1255, 251) if ch == 2008 else
                  (502, 502, 502, 502))
    nfull = nch - sum(taper) // ch
    assert sum(taper) % ch == 0
    spans_mi = [(c * ch, ch) for c in range(nfull)]
    tail_lo = nfull * ch
    for w_ in taper:
        spans_mi.append((tail_lo, w_))
        tail_lo += w_
    assert tail_lo == V
    nsp = len(spans_mi)

    nc = nc_mod
    f32 = mybir.dt.float32
    i32 = mybir.dt.int32
    u32 = mybir.dt.uint32
    Op = mybir.AluOpType
    AX = mybir.AxisListType

    # partition groups covering the cap rows.  The LAST group carries the
    # terminal tail, so it uses baseline per-chunk max_index (short tail, no
    # re-read) -- affordable only when its DMA per chunk (rows*ch*8B/360GBps)
    # outpaces the ~4.4us/chunk of DVE work, i.e. rows >= ~100.  Small groups
    # use max-only streaming with a winning-chunk re-read; their longer tail
    # hides under the next group's stream.
    groups = []  # (row0, rows, mode)
    g1mode = d.get("G1MODE", "maxidx")
    if cap > 128:
        g1rows = min(d.get("G1ROWS", 128), cap)
        g0rows = cap - g1rows
        if g0rows < 48:
            # very small streaming groups crashed the device (validated down
            # to 48 rows); rebalance and drop to rescan mode if the terminal
            # group gets too small for inline max_index to keep DVE fed
            g0rows = 48
            g1rows = cap - g0rows
        mode1 = g1mode if g1rows >= 100 else "rescan"
        groups.append((0, g0rows, "rescan"))
        groups.append((g0rows, g1rows, mode1))
    else:
        groups.append((0, cap, g1mode if cap >= 128 else "rescan"))

    lg_h = nc.dram_tensor("lg", [cap, V], f32, kind="ExternalInput")
    gm_h = nc.dram_tensor("gm", [cap, V], f32, kind="ExternalInput")
    mask_h = nc.dram_tensor("mask", [2 * L, 1], i32, kind="ExternalInput")
    psg_h = nc.dram_tensor("psg", [2 * L, 1], i32, kind="ExternalInput")
    wte_h = nc.dram_tensor("wte", [V, E], f32, kind="ExternalInput")
    wpre_h = nc.dram_tensor("wpre", [V, E], f32, kind="ExternalInput")
    tri_h = nc.dram_tensor("tri", [L, L], f32, kind="ExternalInput")
    psgout_h = nc.dram_tensor("psg_out", [2 * L, E], f32, kind="ExternalOutput")
    tokout_h = nc.dram_tensor("tok_out", [cap, E], f32, kind="ExternalOutput")

    # fine-sliced row views for the data-dependent winning-slot re-read
    lg_v = lg_h[:].rearrange("r (c w) -> (r c) w", w=sw)
    gm_v = gm_h[:].rearrange("r (c w) -> (r c) w", w=sw)

    with tile.TileContext(nc) as tc:
        with (
            tc.tile_pool(name="lpool", bufs=lbufs) as lpool,
            tc.tile_pool(name="lpool1", bufs=d.get("LBUFS1", 5)) as lpool1,
            tc.tile_pool(name="stats", bufs=2) as stats,
            tc.tile_pool(name="small", bufs=2) as small,
            tc.tile_pool(name="emb", bufs=2) as emb,
            tc.tile_pool(name="consts", bufs=1) as consts,
            tc.tile_pool(name="psum", bufs=2, space="PSUM") as psum,
        ):
            # ---- per-core constants (built once) ----
            ones_mat = consts.tile([L, L], f32)
            nc.vector.memset(ones_mat[:], 1.0)
            iota_p_i = consts.tile([L, 1], i32)
            nc.gpsimd.iota(iota_p_i[:], pattern=[[1, 1]], base=0, channel_multiplier=1)
            iota_p = consts.tile([L, 1], f32)
            nc.vector.tensor_copy(out=iota_p[:], in_=iota_p_i[:])

            # tri[i,j] = (j >= i) built on device: one less DMA
            tri_sb = consts.tile([L, L], f32)
            if d.get("TRI_DEV", True):
                trij_i = consts.tile([L, L], i32)
                nc.gpsimd.iota(trij_i[:], pattern=[[1, L]], base=0, channel_multiplier=0)
                trij = consts.tile([L, L], f32)
                nc.vector.tensor_copy(out=trij[:], in_=trij_i[:])
                nc.vector.tensor_scalar(tri_sb[:], trij[:], iota_p[:, 0:1], None, op0=Op.is_ge)
            else:
                nc.gpsimd.dma_start(out=tri_sb[:], in_=tri_h[:])

            need_mi = any(m == "maxidx" for (_, _, m) in groups)
            need_rs = any(m == "rescan" for (_, _, m) in groups)
            if need_mi:
                iotas_i = consts.tile([L, nsp], i32)
                nc.gpsimd.iota(iotas_i[:], pattern=[[1, nsp]], base=0, channel_multiplier=0)
                iotas = consts.tile([L, nsp], f32)
                nc.vector.tensor_copy(out=iotas[:], in_=iotas_i[:])
                # srev[c] = nsp - c (used to pick the LOWEST span attaining the max)
                srev = consts.tile([L, nsp], f32)
                nc.vector.tensor_scalar(srev[:], iotas[:], -1.0, float(nsp), op0=Op.mult, op1=Op.add)
                # per-span start offsets (hot = bases[c*] + within-span index)
                bases = consts.tile([L, nsp], f32)
                nc.vector.tensor_scalar(bases[:], iotas[:], float(ch), None, op0=Op.mult)
                for ci, (lo_c, _w) in enumerate(spans_mi):
                    if lo_c != ci * ch:
                        nc.vector.memset(bases[:, ci:ci + 1], float(lo_c))
            if need_rs:
                iotaf_i = consts.tile([L, nfine], i32)
                nc.gpsimd.iota(iotaf_i[:], pattern=[[1, nfine]], base=0, channel_multiplier=0)
                iotaf = consts.tile([L, nfine], f32)
                nc.vector.tensor_copy(out=iotaf[:], in_=iotaf_i[:])
                frev = consts.tile([L, nfine], f32)
                nc.vector.tensor_scalar(frev[:], iotaf[:], -1.0, float(nfine), op0=Op.mult, op1=Op.add)
            # per-partition fine-sliced row base: (row + group offset) * nfine
            rowb = {}
            for (g0_, _gp, mode) in groups:
                if mode != "rescan":
                    continue
                rb = consts.tile([L, 1], f32)
                nc.vector.tensor_scalar(rb[:], iota_p[:], float(nfine), float(g0_ * nfine),
                                        op0=Op.mult, op1=Op.add)
                rowb[g0_] = rb

            psg_state = {}

            def psg_a(t):
                """psg stage 1: mask load, length, gather indices; issues the
                mask/psg row gathers.  All DVE ops here depend only on the
                mask load + ones matmul, so they run during kernel startup."""
                tok = slice(t * L, (t + 1) * L)
                mask_i = small.tile([L, 1], i32, tag="mask_i")
                nc.scalar.dma_start(out=mask_i[:], in_=mask_h[tok, :])
                mask_f = small.tile([L, 1], f32, tag="mask_f")
                nc.vector.tensor_copy(out=mask_f[:], in_=mask_i[:])

                # s (broadcast to all partitions) = sum(mask) via ones matmul
                s_ps = psum.tile([L, 1], f32, tag="s_ps")
                nc.tensor.matmul(out=s_ps[:], lhsT=ones_mat[:], rhs=mask_f[:], start=True, stop=True)
                psg_state[t] = dict(s_ps=s_ps)

            def psg_a2(t):
                """psg stage 1b: index arithmetic (the cold-PE matmul from
                stage 1a is long done, so nothing here stalls the DVE queue);
                issues the mask/psg row gathers."""
                st = psg_state[t]
                s_ps = st["s_ps"]
                s_bc = small.tile([L, 1], f32, tag="s_bc")
                nc.vector.tensor_copy(out=s_bc[:], in_=s_ps[:])

                def mod_l(x_ap, lo_fix=True, hi_fix=True, tagp=""):
                    # x <- x mod L for x in (-L, 2L)
                    if hi_fix:
                        ge = small.tile([L, 1], f32, tag="ge" + tagp)
                        nc.vector.tensor_scalar(ge[:], x_ap, float(L), None, op0=Op.is_ge)
                        nc.vector.scalar_tensor_tensor(
                            out=x_ap, in0=ge[:], scalar=-float(L), in1=x_ap, op0=Op.mult, op1=Op.add)
                    if lo_fix:
                        lt_ = small.tile([L, 1], f32, tag="lt" + tagp)
                        nc.vector.tensor_scalar(lt_[:], x_ap, 0.0, None, op0=Op.is_lt)
                        nc.vector.scalar_tensor_tensor(
                            out=x_ap, in0=lt_[:], scalar=float(L), in1=x_ap, op0=Op.mult, op1=Op.add)

                # fidx = (L-1 + s - l) mod L   (flipped-mask gather index)
                fidx = small.tile([L, 1], f32, tag="fidx")
                nc.vector.scalar_tensor_tensor(
                    out=fidx[:], in0=s_bc[:], scalar=float(L - 1), in1=iota_p[:],
                    op0=Op.add, op1=Op.subtract)
                mod_l(fidx[:], lo_fix=False, tagp="f")

                # pidx = (L-1 - s + l) mod L   (rolled-psg gather index)
                pidx = small.tile([L, 1], f32, tag="pidx")
                nc.vector.scalar_tensor_tensor(
                    out=pidx[:], in0=s_bc[:], scalar=-1.0, in1=iota_p[:],
                    op0=Op.mult, op1=Op.add)
                nc.vector.tensor_scalar(pidx[:], pidx[:], float(L - 1), None, op0=Op.add)
                mod_l(pidx[:], tagp="p")

                # k = (l - s) mod L ; BOS position is k == 0
                kk = small.tile([L, 1], f32, tag="kk")
                nc.vector.scalar_tensor_tensor(
                    out=kk[:], in0=s_bc[:], scalar=-1.0, in1=iota_p[:],
                    op0=Op.mult, op1=Op.add)
                mod_l(kk[:], hi_fix=False, tagp="k")
                bos = small.tile([L, 1], f32, tag="bos")
                nc.vector.tensor_scalar(bos[:], kk[:], 0.0, None, op0=Op.is_equal)

                # gather mask[fidx] and psg[pidx] (within this batch row)
                fr_i = small.tile([L, 1], i32, tag="fr_i")
                nc.vector.tensor_scalar(fidx[:], fidx[:], float(t * L), None, op0=Op.add)
                nc.vector.tensor_copy(out=fr_i[:], in_=fidx[:])
                mrev = small.tile([L, 1], i32, tag="mrev")
                nc.gpsimd.indirect_dma_start(
                    out=mrev[:], out_offset=None, in_=mask_h[:],
                    in_offset=IndirectOffsetOnAxis(ap=fr_i[:, 0:1], axis=0),
                )
                pr_i = small.tile([L, 1], i32, tag="pr_i")
                nc.vector.tensor_scalar(pidx[:], pidx[:], float(t * L), None, op0=Op.add)
                nc.vector.tensor_copy(out=pr_i[:], in_=pidx[:])
                prot = small.tile([L, 1], i32, tag="prot")
                nc.gpsimd.indirect_dma_start(
                    out=prot[:], out_offset=None, in_=psg_h[:],
                    in_offset=IndirectOffsetOnAxis(ap=pr_i[:, 0:1], axis=0),
                )
                st.update(bos=bos, mrev=mrev, prot=prot)

            def psg_b(t):
                """psg stage 2: combine the gathered rows (long since arrived)
                into trunc/flag; issues the psg embedding gather."""
                st = psg_state[t]
                bos, mrev, prot = st["bos"], st["mrev"], st["prot"]
                # f_rot = 1 - mask[fidx]
                mrev_f = small.tile([L, 1], f32, tag="mrev_f")
                nc.vector.tensor_copy(out=mrev_f[:], in_=mrev[:])
                frot = small.tile([L, 1], f32, tag="frot")
                nc.vector.tensor_scalar(frot[:], mrev_f[:], -1.0, 1.0, op0=Op.mult, op1=Op.add)
                # psg_rot = bos ? 1 : psg[pidx]
                prot_f = small.tile([L, 1], f32, tag="prot_f")
                nc.vector.tensor_copy(out=prot_f[:], in_=prot[:])
                nbos = small.tile([L, 1], f32, tag="nbos")
                nc.vector.tensor_scalar(nbos[:], bos[:], -1.0, 1.0, op0=Op.mult, op1=Op.add)
                nc.vector.tensor_tensor(out=prot_f[:], in0=prot_f[:], in1=nbos[:], op=Op.mult)
                nc.vector.tensor_tensor(out=prot_f[:], in0=prot_f[:], in1=bos[:], op=Op.add)
                # trunc = f_rot * psg_rot
                trunc = small.tile([L, 1], f32, tag="trunc")
                nc.vector.tensor_tensor(out=trunc[:], in0=frot[:], in1=prot_f[:], op=Op.mult)

                # flag = cumsum(trunc != 0) > 0 via triangular matmul
                nz = small.tile([L, 1], f32, tag="nz")
                nc.vector.tensor_scalar(nz[:], trunc[:], 0.0, None, op0=Op.not_equal)
                cnt_ps = psum.tile([L, 1], f32, tag="cnt_ps")
                nc.tensor.matmul(out=cnt_ps[:], lhsT=tri_sb[:], rhs=nz[:], start=True, stop=True)

                trunc_i = small.tile([L, 1], i32, tag="trunc_i")
                nc.vector.tensor_copy(out=trunc_i[:], in_=trunc[:])
                psgemb = emb.tile([L, E], f32, tag="psgemb")
                nc.gpsimd.indirect_dma_start(
                    out=psgemb[:], out_offset=None, in_=wte_h[:],
                    in_offset=IndirectOffsetOnAxis(ap=trunc_i[:, 0:1], axis=0),
                )
                st["cnt_ps"] = cnt_ps
                st["psgemb"] = psgemb

            def psg_c(t):
                """psg stage 3: flag-mask the embeddings and store."""
                st = psg_state[t]
                tok = slice(t * L, (t + 1) * L)
                flag = small.tile([L, 1], f32, tag="flag")
                nc.vector.tensor_scalar(flag[:], st["cnt_ps"][:], 0.0, None, op0=Op.is_gt)
                outp = emb.tile([L, E], f32, tag="outp")
                nc.vector.tensor_scalar(outp[:], st["psgemb"][:], flag[:, 0:1],
                                        None, op0=Op.mult)
                nc.sync.dma_start(out=psgout_h[tok, :], in_=outp[:])

            def stream_group(g0_, gp, mode, interleave=None):
                """DMA-bound pass over the vocab for rows [g0_, g0_+gp):
                per span, load logits, accumulate gumbel in the DMA
                datapath, then track span maxes (and, in maxidx mode, the
                first within-span argmax position).  `interleave` is the
                prior group's deferred rescan finish, emitted a few chunks
                in -- by then its re-read data has arrived, so the max_index
                never stalls the in-order DVE queue."""
                rows = slice(g0_, g0_ + gp)
                idx_all = None
                if mode == "maxidx":
                    m_all = stats.tile([L, nsp], f32, tag="m_all" + mode)
                    idx_all = stats.tile([L, nsp], f32, tag="idx_all")
                    spans = spans_mi
                else:
                    m_all = stats.tile([L, nfine], f32, tag="m_all" + mode)
                    spans = [(c * ch, ch) for c in range(nch)]
                pool = lpool if g0_ == 0 else lpool1
                for c, (lo, w) in enumerate(spans):
                    for fn in (interleave or {}).get(c, []):
                        fn()
                    lt = pool.tile([L, ch], f32, tag="lt")
                    ldeng = nc.scalar if c % 2 else nc.sync
                    ldeng.dma_start(out=lt[0:gp, 0:w], in_=lg_h[rows, lo:lo + w])
                    nc.gpsimd.dma_start(
                        out=lt[0:gp, 0:w], in_=gm_h[rows, lo:lo + w], accum_op=Op.add)
                    if mode == "maxidx":
                        mx8 = small.tile([L, 8], f32, tag="mx8")
                        nc.vector.max(out=mx8[0:gp, :], in_=lt[0:gp, 0:w])
                        nc.vector.tensor_copy(out=m_all[0:gp, c:c + 1], in_=mx8[0:gp, 0:1])
                        mi8 = small.tile([L, 8], u32, tag="mi8s")
                        nc.vector.max_index(out=mi8[0:gp, :], in_max=mx8[0:gp, :],
                                            in_values=lt[0:gp, 0:w])
                        nc.vector.tensor_copy(out=idx_all[0:gp, c:c + 1], in_=mi8[0:gp, 0:1])
                    else:
                        # chunk maxes at sub-chunk (sw-wide) granularity
                        lt3 = lt[0:gp, 0:w].rearrange("p (a b) -> p a b", b=sw)
                        nc.vector.reduce_max(
                            out=m_all[0:gp, c * SUB:(c + 1) * SUB], in_=lt3, axis=AX.X)
                return m_all, idx_all

            def tournament(g0_, gp, m_all, rev, nslots):
                """global max + LOWEST slot attaining it (argmax tie rule)."""
                gmax = small.tile([L, 1], f32, tag="gmax")
                nc.vector.reduce_max(out=gmax[0:gp, :], in_=m_all[0:gp, :], axis=AX.X)
                sel = small.tile([L, nslots], f32, tag="sel" + str(nslots))
                nc.vector.scalar_tensor_tensor(
                    out=sel[0:gp, :], in0=m_all[0:gp, :], scalar=gmax[0:gp, 0:1],
                    in1=rev[0:gp, :], op0=Op.is_ge, op1=Op.mult)
                cmax = small.tile([L, 1], f32, tag="cmax")
                nc.vector.reduce_max(out=cmax[0:gp, :], in_=sel[0:gp, :], axis=AX.X)
                cstar = small.tile([L, 1], f32, tag="cstar")
                nc.vector.tensor_scalar(cstar[0:gp, :], cmax[0:gp, :], -1.0, float(nslots),
                                        op0=Op.mult, op1=Op.add)
                return gmax, cstar

            def finish_group(g0_, gp, hot_f, split=False):
                """hot -> W_pre gather -> tok_out rows.  split pipelines the
                gather/store pair in two row halves (shorter terminal chain)."""
                hot_i = small.tile([L, 1], i32, tag="hot_i")
                nc.vector.tensor_copy(out=hot_i[0:gp, :], in_=hot_f[0:gp, :])
                if gp == 1:
                    # single-partition indirect DMAs are rejected; pad with a
                    # zeroed second index and gather one junk row (never read)
                    nc.vector.memset(hot_i[1:2, :], 0)
                halves = [(0, gp)] if not split or gp < 32 else [
                    (0, gp // 2), (gp // 2, gp)]
                tokemb = emb.tile([L, E], f32, tag="tokemb")
                for (a, b) in halves:
                    bi = 2 if (b - a == 1 and a == 0) else b
                    nc.gpsimd.indirect_dma_start(
                        out=tokemb[a:bi, :], out_offset=None, in_=wpre_h[:],
                        in_offset=IndirectOffsetOnAxis(ap=hot_i[a:bi, 0:1], axis=0))
                steng = {"swdge": nc.gpsimd, "act": nc.scalar}.get(
                    d.get("STORE_ENG", "sp"), nc.sync)
                for (a, b) in halves:
                    steng.dma_start(out=tokout_h[g0_ + a:g0_ + b, :], in_=tokemb[a:b, :])

            def tail_rescan_start(g0_, gp, m_all):
                """Tournament (all data ready -> no DVE stalls) + issue the
                winning-sub-chunk re-read. The dependent max_index is deferred
                (tail_rescan_end) so its DMA wait never blocks the in-order
                DVE queue while the next group streams."""
                gmax, sstar = tournament(g0_, gp, m_all, frev, nfine)
                # fine-sliced row index for the re-read: (row + g0)*nfine + sstar
                ridx_f = small.tile([L, 1], f32, tag="ridx_f")
                nc.vector.tensor_tensor(out=ridx_f[0:gp, :], in0=rowb[g0_][0:gp, :],
                                        in1=sstar[0:gp, :], op=Op.add)
                ridx_i = small.tile([L, 1], i32, tag="ridx_i")
                nc.vector.tensor_copy(out=ridx_i[0:gp, :], in_=ridx_f[0:gp, :])
                gmax8 = small.tile([L, 8], f32, tag="gmax8")
                nc.vector.tensor_copy(out=gmax8[0:gp, :],
                                      in_=gmax[0:gp, 0:1].to_broadcast([gp, 8]))

                # single-partition indirect DMAs are rejected; pad the index
                # tile with a zeroed row and gather one junk row (never read)
                gpi = 2 if gp == 1 else gp
                if gpi != gp:
                    nc.vector.memset(ridx_i[gp:gpi, :], 0)

                # re-read ONLY the winning sub-chunk (logits + CCE-add gumbel);
                # bit-matches the stream pass, so gmax is found exactly
                rs = emb.tile([L, sw], f32, tag="rs")
                nc.gpsimd.indirect_dma_start(
                    out=rs[0:gpi, :], out_offset=None, in_=lg_v,
                    in_offset=IndirectOffsetOnAxis(ap=ridx_i[0:gpi, 0:1], axis=0))
                nc.gpsimd.indirect_dma_start(
                    out=rs[0:gpi, :], out_offset=None, in_=gm_v,
                    in_offset=IndirectOffsetOnAxis(ap=ridx_i[0:gpi, 0:1], axis=0),
                    compute_op=Op.add)
                return sstar, gmax8, rs

            def tail_rescan_end(g0_, gp, state):
                sstar, gmax8, rs = state
                mi8 = small.tile([L, 8], u32, tag="mi8")
                nc.vector.max_index(out=mi8[0:gp, :], in_max=gmax8[0:gp, :],
                                    in_values=rs[0:gp, :])
                mi_f = small.tile([L, 1], f32, tag="mi_f")
                nc.vector.tensor_copy(out=mi_f[0:gp, :], in_=mi8[0:gp, 0:1])
                hot_f = small.tile([L, 1], f32, tag="hot_f")
                nc.vector.scalar_tensor_tensor(
                    out=hot_f[0:gp, :], in0=sstar[0:gp, :], scalar=float(sw),
                    in1=mi_f[0:gp, :], op0=Op.mult, op1=Op.add)
                finish_group(g0_, gp, hot_f)

            def tail_maxidx(g0_, gp, m_all, idx_all):
                # bidx[c] = span base + within-span argmax = the hot candidate
                # of span c; computed per span during the stream (off the
                # terminal critical path)
                bidx = stats.tile([L, nsp], f32, tag="bidx")
                nc.vector.tensor_tensor(out=bidx[0:gp, :], in0=bases[0:gp, :],
                                        in1=idx_all[0:gp, :], op=Op.add)
                _gmax, cstar = tournament(g0_, gp, m_all, srev, nsp)
                # select the winning span's candidate: non-winning columns
                # zero out, the winner holds hot >= 0
                junk = small.tile([L, nsp], f32, tag="junk")
                nc.vector.scalar_tensor_tensor(
                    out=junk[0:gp, :], in0=iotas[0:gp, :], scalar=cstar[0:gp, 0:1],
                    in1=bidx[0:gp, :], op0=Op.is_equal, op1=Op.mult)
                hot_f = small.tile([L, 1], f32, tag="hot_f")
                nc.vector.reduce_max(out=hot_f[0:gp, :], in_=junk[0:gp, :], axis=AX.X)
                finish_group(g0_, gp, hot_f, split=d.get("WSPLIT", False))

            # psg stage 1a first: only the mask load + cold-PE matmul issue,
            # so the DVE queue stays clear for the first chunk maxes
            for t in range(2):
                psg_a(t)
            pending = {d.get("ILA", 4): [lambda: [psg_a2(t) for t in range(2)]]}
            for gi, (g0_, gp, mode) in enumerate(groups):
                last = gi == len(groups) - 1
                m_all, idx_all = stream_group(g0_, gp, mode,
                                              interleave=pending)
                pending = {}
                if not last:
                    # non-terminal group: tournament + issue the re-read now;
                    # the dependent finish and psg stages 2/3 slot a few
                    # chunks into the next group's stream (data ready by then)
                    state = tail_rescan_start(g0_, gp, m_all)
                    for t in range(2):
                        psg_b(t)
                    pending = {
                        d.get("ILB", 13): [
                            lambda g0=g0_, gp_=gp, st=state: tail_rescan_end(g0, gp_, st)],
                        d.get("ILC", 8): [lambda: [psg_c(t) for t in range(2)]],
                    }
                else:
                    if len(groups) == 1:
                        for t in range(2):
                            psg_b(t)
                    if mode == "maxidx":
                        tail_maxidx(g0_, gp, m_all, idx_all)
                    else:
                        tail_rescan_end(g0_, gp, tail_rescan_start(g0_, gp, m_all))
                    if len(groups) == 1:
                        for t in range(2):
                            psg_c(t)

    return nc


_BUILD_CACHE = {}


def _get_module_v2(cap, dims_key=None, dims=None):
    key = ("v2", cap, dims_key)
    if key not in _BUILD_CACHE:
        import concourse.bacc as bacc

        nc = bacc.Bacc("TRN2", target_bir_lowering=False, debug=False)
        _build_v2(nc, cap, dims)
        nc.compile()
        _BUILD_CACHE[key] = nc
    return _BUILD_CACHE[key]


_ROWMAP_CACHE = {}


def _nearest_maps():
    """Replicate the reference's f32 grid_sample-nearest index maps with jnp
    on the same backend the reference runs on (bit-exact by construction)."""
    if "maps" not in _ROWMAP_CACHE:
        import jax.numpy as jnp

        def nearest(size):
            lin = jnp.linspace(-1.0, 1.0, size)
            ix = ((lin + 1.0) * size - 1.0) / 2.0
            return np.asarray(jnp.clip(jnp.round(ix), 0, size - 1).astype(jnp.int32))

        _ROWMAP_CACHE["maps"] = (nearest(V), nearest(E))
    return _ROWMAP_CACHE["maps"]


_TRI = None

# test/dev hooks: set TRACE=True before calling kernel() to capture an NTFF
# profile; the BassKernelResults of the last run is stored in LAST_RESULT.
TRACE = False
LAST_RESULT = None
LAST_MODULE = None


def kernel(logits, rwrt_attention_mask, psg_input_ids, word_embeddings, gumbel_noise):
    from concourse.bass_utils import run_bass_kernel_spmd

    global _TRI, LAST_RESULT, LAST_MODULE
    logits = np.ascontiguousarray(np.asarray(logits, dtype=np.float32))
    gumbel = np.ascontiguousarray(np.asarray(gumbel_noise, dtype=np.float32))
    mask = np.ascontiguousarray(np.asarray(rwrt_attention_mask, dtype=np.int32))
    psg = np.ascontiguousarray(np.asarray(psg_input_ids, dtype=np.int32))
    wte = np.ascontiguousarray(np.asarray(word_embeddings, dtype=np.float32))

    rowmap, colmap = _nearest_maps()
    # token branch reads W[rowmap][:, colmap]; constant-fold both static maps
    wpre = np.ascontiguousarray(wte[rowmap][:, colmap])
    if _TRI is None:
        _TRI = np.ascontiguousarray(np.triu(np.ones((L, L), dtype=np.float32)))

    # ---- sharding: compact the active (mask==1) tokens across cores ----
    flat_mask = mask.reshape(B * L) != 0
    ids = np.nonzero(flat_mask)[0].astype(np.int64)    # batch-major order
    n_act = int(ids.size)
    cap = max(8, -(-n_act // N_CORES))                 # tokens per core
    cap += cap & 1     # even row counts only: an odd-cap module (odd-row
                       # groups / odd-row tok_out) hit NRT_EXEC_UNIT_
                       # UNRECOVERABLE on device; one pad slot is free
    pad_id = int(ids[-1]) if n_act else 0
    ids_p = np.full(N_CORES * cap, pad_id, dtype=np.int64)
    ids_p[:n_act] = ids
    lg_flat = logits.reshape(B * L, V)
    gm_flat = gumbel.reshape(B * L, V)

    nc = _get_module_v2(cap)
    LAST_MODULE = nc

    in_maps = []
    for m in range(N_CORES):
        sl = ids_p[m * cap:(m + 1) * cap]
        im = {
            "lg": np.ascontiguousarray(lg_flat[sl]),
            "gm": np.ascontiguousarray(gm_flat[sl]),
            "mask": mask[2 * m:2 * m + 2].reshape(2 * L, 1),
            "psg": psg[2 * m:2 * m + 2].reshape(2 * L, 1),
            "wte": wte,
            "wpre": wpre,
            "tri": _TRI,
        }
        in_maps.append(im)

    try:
        LAST_RESULT = run_bass_kernel_spmd(nc, in_maps, list(range(N_CORES)), trace=TRACE)
    except Exception:
        # the axon-relayed device occasionally reports a transient
        # NRT_EXEC_UNIT_UNRECOVERABLE; re-runs (with backoff) recover it
        import time as _time

        for backoff in (2.0, 15.0):
            _time.sleep(backoff)
            try:
                LAST_RESULT = run_bass_kernel_spmd(
                    nc, in_maps, list(range(N_CORES)), trace=TRACE)
                break
            except Exception:
                if backoff == 15.0:
                    raise
    res = LAST_RESULT.results

    # ---- unshard: psg partial everywhere + scatter-add the token rows ----
    out = np.concatenate(
        [res[m]["psg_out"].reshape(2, L, E) for m in range(N_CORES)], axis=0)
    if n_act:
        tok = np.concatenate(
            [res[m]["tok_out"].reshape(cap, E) for m in range(N_CORES)], axis=0)
        flat = out.reshape(B * L, E)
        flat[ids] += tok[:n_act]
    return out


# revision 41
# speedup vs baseline: 1.0060x; 1.0009x over previous
"""Trainium2 Bass kernel: gumbel-softmax-argmax embedding lookup (end-to-end).

Reference math (nn_End2End_49495203119139):
    hot  = argmax_V(softmax((logits + gumbel)/tau))       == argmax_V(logits+gumbel)
    row  = grid_sample-nearest index map of hot            == ROWMAP[hot]  (LUT)
    tok_emb = W[row][:, col_map]   with col_map == arange(E)  (verified at runtime)
    inputs_embeds = tok_emb * mask
    psg_roll = roll(psg_ids, 1, axis=1); psg_roll[:,0] = 1
    extr  = (1 - mask[:, ::-1]) * psg_roll
    trunc = rotate_right(extr, shifts) with shifts = mask.sum(-1)   (per row)
    flag  = cumsum(trunc != 0, -1) > 0
    out   = inputs_embeds + where(flag, W[trunc], 0)

Key observation: inputs_embeds is multiplied by the attention mask, so the
argmax over the 32128-wide vocab -- the part that forces streaming the
526 MB of logits+gumbel -- is only needed for tokens with mask == 1.
The mask is an input, so sharding can compact: the host flattens the
active (b, l) positions (N of B*L), pads to C = ceil(N/8) per core, and
ships each core a compacted [C, V] slice of logits and gumbel.  Each core
streams only its C rows (vs B*L/8 = 256 for dense data-parallel).  The
passage branch (cheap) stays on the owner core of each batch row; the two
partial outputs (psg_out for all positions, tok_out rows for the active
tokens) are combined by the host during the unshard step -- an exact f32
add of the same values the dense kernel would have added on-device.

Per-core device plan (memory-bound part = streaming the compacted
logits+gumbel, 2*C*V*4 bytes ~= 45.8 MB for C=178, ~127 us HBM floor at
360 GB/s per core).  Rows go in two partition groups, (C-128, 128); per
group and per vocab span: HWDGE-load the logits span (alternating SP/ACT
DMA queues), add the gumbel span with one SWDGE inline-accumulate DMA
(CCE add; descriptors must stay <= 2048 elements), then find the span
max on DVE.  The argmax strategy differs per group because DVE is the
scarce engine after compaction (max+max_index of every span would put
~145 us on DVE vs the ~135 us DMA floor):
  - FIRST group (C-128 rows): max-only streaming -- one reduce_max per
    span at 251-element sub-span granularity.  A short tournament picks
    the LOWEST sub-span attaining the global max (argmax tie rule), then
    that sub-span alone is re-read (indirect row gather + CCE-add) and
    ONE max_index against the broadcast global max finds the in-span
    position -- exact because the CCE re-add bit-matches the stream
    pass.  Its tail hides under the second group's stream; the max_index
    is deferred a few spans into it so the re-read's DMA latency never
    blocks the in-order DVE queue.
  - LAST group (128 rows; carries the terminal tail): per-span max +
    max_index inline (affordable: its 5.7 us/span DMA outpaces the
    4.4 us/span DVE cost), so after the final span only the tournament
    and the embedding gather remain.  The vocab end is tapered into
    shrinking spans so the final span's DVE work is short.
  - hot -> token embedding via ONE indirect gather from a host-built
    table W_pre = W[rowmap][:, colmap] (constant-folds the grid_sample
    LUTs, removing a dependent gather from the critical tail).
  - the passage branch is pure index arithmetic on [128,1] tiles: the
    reverse/roll/rotate are folded into gather indices modulo L, the
    mask-sum and cumsum are 0/1 matmuls against ones/triangular matrices
    (exact in any PE precision).  psg_out = flag * W[trunc] is stored for
    every position.  It is emitted in three stages (index math + row
    gathers up front, combine after the first group's stream, flag+store
    mid second stream) so its cross-engine waits never stall the DVE
    queue ahead of stream work.
"""

import numpy as np

B = 16
L = 128
V = 32128
E = 768
N_CORES = 8
CH = 2008                     # vocab chunk (free dim) per streamed tile;
                              # <= 2048 elements so a gumbel chunk is ONE
                              # CCE-add DMA descriptor per partition row
NCH = V // CH                 # 16 chunks


def _build_v2(nc_mod, cap, dims=None):
    """Per-core module for the compacted layout. cap = tokens per core."""
    import concourse.tile as tile
    from concourse import bass, mybir
    from concourse.bass import IndirectOffsetOnAxis

    d = dims or {}
    ch = d.get("CH", CH)
    nch = V // ch
    lbufs = d.get("LBUFS", 10)
    # rescan-mode groups track chunk maxes at sub-chunk granularity so the
    # winning-chunk re-read (and its max_index) touch only SW elements
    SUB = ch // 251
    sw = 251                      # V == nch*SUB*sw exactly
    nfine = nch * SUB             # 128 fine slots
    # maxidx-mode groups taper the end of the vocab so the last spans'
    # (max + max_index) DVE work after the final CCE-add is short
    taper = d.get("TAPER", (1255, 1255, 1255, 251) if ch == 2008 else
                  (502, 502, 502, 502))
    nfull = nch - sum(taper) // ch
    assert sum(taper) % ch == 0
    spans_mi = [(c * ch, ch) for c in range(nfull)]
    tail_lo = nfull * ch
    for w_ in taper:
        spans_mi.append((tail_lo, w_))
        tail_lo += w_
    assert tail_lo == V
    nsp = len(spans_mi)

    nc = nc_mod
    f32 = mybir.dt.float32
    i32 = mybir.dt.int32
    u32 = mybir.dt.uint32
    Op = mybir.AluOpType
    AX = mybir.AxisListType

    # partition groups covering the cap rows.  The LAST group carries the
    # terminal tail, so it uses baseline per-chunk max_index (short tail, no
    # re-read) -- affordable only when its DMA per chunk (rows*ch*8B/360GBps)
    # outpaces the ~4.4us/chunk of DVE work, i.e. rows >= ~100.  Small groups
    # use max-only streaming with a winning-chunk re-read; their longer tail
    # hides under the next group's stream.
    groups = []  # (row0, rows, mode)
    g1mode = d.get("G1MODE", "maxidx")
    if cap > 128:
        g1rows = min(d.get("G1ROWS", 128), cap)
        g0rows = cap - g1rows
        if g0rows < 48:
            # very small streaming groups crashed the device (validated down
            # to 48 rows); rebalance and drop to rescan mode if the terminal
            # group gets too small for inline max_index to keep DVE fed
            g0rows = 48
            g1rows = cap - g0rows
        mode1 = g1mode if g1rows >= 100 else "rescan"
        groups.append((0, g0rows, "rescan"))
        groups.append((g0rows, g1rows, mode1))
    else:
        groups.append((0, cap, g1mode if cap >= 128 else "rescan"))

    lg_h = nc.dram_tensor("lg", [cap, V], f32, kind="ExternalInput")
    gm_h = nc.dram_tensor("gm", [cap, V], f32, kind="ExternalInput")
    mask_h = nc.dram_tensor("mask", [2 * L, 1], i32, kind="ExternalInput")
    psg_h = nc.dram_tensor("psg", [2 * L, 1], i32, kind="ExternalInput")
    wte_h = nc.dram_tensor("wte", [V, E], f32, kind="ExternalInput")
    wpre_h = nc.dram_tensor("wpre", [V, E], f32, kind="ExternalInput")
    tri_h = nc.dram_tensor("tri", [L, L], f32, kind="ExternalInput")
    psgout_h = nc.dram_tensor("psg_out", [2 * L, E], f32, kind="ExternalOutput")
    tokout_h = nc.dram_tensor("tok_out", [cap, E], f32, kind="ExternalOutput")

    # fine-sliced row views for the data-dependent winning-slot re-read
    lg_v = lg_h[:].rearrange("r (c w) -> (r c) w", w=sw)
    gm_v = gm_h[:].rearrange("r (c w) -> (r c) w", w=sw)

    with tile.TileContext(nc) as tc:
        with (
            tc.tile_pool(name="lpool", bufs=lbufs) as lpool,
            tc.tile_pool(name="lpool1", bufs=d.get("LBUFS1", 5)) as lpool1,
            tc.tile_pool(name="stats", bufs=2) as stats,
            tc.tile_pool(name="small", bufs=2) as small,
            tc.tile_pool(name="emb", bufs=2) as emb,
            tc.tile_pool(name="consts", bufs=1) as consts,
            tc.tile_pool(name="psum", bufs=2, space="PSUM") as psum,
        ):
            # ---- per-core constants (built once) ----
            ones_mat = consts.tile([L, L], f32)
            nc.vector.memset(ones_mat[:], 1.0)
            iota_p_i = consts.tile([L, 1], i32)
            nc.gpsimd.iota(iota_p_i[:], pattern=[[1, 1]], base=0, channel_multiplier=1)
            iota_p = consts.tile([L, 1], f32)
            nc.vector.tensor_copy(out=iota_p[:], in_=iota_p_i[:])

            # tri[i,j] = (j >= i) built on device: one less DMA
            tri_sb = consts.tile([L, L], f32)
            if d.get("TRI_DEV", True):
                trij_i = consts.tile([L, L], i32)
                nc.gpsimd.iota(trij_i[:], pattern=[[1, L]], base=0, channel_multiplier=0)
                trij = consts.tile([L, L], f32)
                nc.vector.tensor_copy(out=trij[:], in_=trij_i[:])
                nc.vector.tensor_scalar(tri_sb[:], trij[:], iota_p[:, 0:1], None, op0=Op.is_ge)
            else:
                nc.gpsimd.dma_start(out=tri_sb[:], in_=tri_h[:])

            need_mi = any(m == "maxidx" for (_, _, m) in groups)
            need_rs = any(m == "rescan" for (_, _, m) in groups)
            if need_mi:
                iotas_i = consts.tile([L, nsp], i32)
                nc.gpsimd.iota(iotas_i[:], pattern=[[1, nsp]], base=0, channel_multiplier=0)
                iotas = consts.tile([L, nsp], f32)
                nc.vector.tensor_copy(out=iotas[:], in_=iotas_i[:])
                # srev[c] = nsp - c (used to pick the LOWEST span attaining the max)
                srev = consts.tile([L, nsp], f32)
                nc.vector.tensor_scalar(srev[:], iotas[:], -1.0, float(nsp), op0=Op.mult, op1=Op.add)
                # per-span start offsets (hot = bases[c*] + within-span index)
                bases = consts.tile([L, nsp], f32)
                nc.vector.tensor_scalar(bases[:], iotas[:], float(ch), None, op0=Op.mult)
                for ci, (lo_c, _w) in enumerate(spans_mi):
                    if lo_c != ci * ch:
                        nc.vector.memset(bases[:, ci:ci + 1], float(lo_c))
            if need_rs:
                iotaf_i = consts.tile([L, nfine], i32)
                nc.gpsimd.iota(iotaf_i[:], pattern=[[1, nfine]], base=0, channel_multiplier=0)
                iotaf = consts.tile([L, nfine], f32)
                nc.vector.tensor_copy(out=iotaf[:], in_=iotaf_i[:])
                frev = consts.tile([L, nfine], f32)
                nc.vector.tensor_scalar(frev[:], iotaf[:], -1.0, float(nfine), op0=Op.mult, op1=Op.add)
            # per-partition fine-sliced row base: (row + group offset) * nfine
            rowb = {}
            for (g0_, _gp, mode) in groups:
                if mode != "rescan":
                    continue
                rb = consts.tile([L, 1], f32)
                nc.vector.tensor_scalar(rb[:], iota_p[:], float(nfine), float(g0_ * nfine),
                                        op0=Op.mult, op1=Op.add)
                rowb[g0_] = rb

            psg_state = {}

            def psg_a(t):
                """psg stage 1: mask load, length, gather indices; issues the
                mask/psg row gathers.  All DVE ops here depend only on the
                mask load + ones matmul, so they run during kernel startup."""
                tok = slice(t * L, (t + 1) * L)
                mask_i = small.tile([L, 1], i32, tag="mask_i")
                nc.scalar.dma_start(out=mask_i[:], in_=mask_h[tok, :])
                mask_f = small.tile([L, 1], f32, tag="mask_f")
                nc.vector.tensor_copy(out=mask_f[:], in_=mask_i[:])

                # s (broadcast to all partitions) = sum(mask) via ones matmul
                s_ps = psum.tile([L, 1], f32, tag="s_ps")
                nc.tensor.matmul(out=s_ps[:], lhsT=ones_mat[:], rhs=mask_f[:], start=True, stop=True)
                psg_state[t] = dict(s_ps=s_ps)

            def psg_a2(t):
                """psg stage 1b: index arithmetic (the cold-PE matmul from
                stage 1a is long done, so nothing here stalls the DVE queue);
                issues the mask/psg row gathers."""
                st = psg_state[t]
                s_ps = st["s_ps"]
                s_bc = small.tile([L, 1], f32, tag="s_bc")
                nc.vector.tensor_copy(out=s_bc[:], in_=s_ps[:])

                def mod_l(x_ap, lo_fix=True, hi_fix=True, tagp=""):
                    # x <- x mod L for x in (-L, 2L)
                    if hi_fix:
                        ge = small.tile([L, 1], f32, tag="ge" + tagp)
                        nc.vector.tensor_scalar(ge[:], x_ap, float(L), None, op0=Op.is_ge)
                        nc.vector.scalar_tensor_tensor(
                            out=x_ap, in0=ge[:], scalar=-float(L), in1=x_ap, op0=Op.mult, op1=Op.add)
                    if lo_fix:
                        lt_ = small.tile([L, 1], f32, tag="lt" + tagp)
                        nc.vector.tensor_scalar(lt_[:], x_ap, 0.0, None, op0=Op.is_lt)
                        nc.vector.scalar_tensor_tensor(
                            out=x_ap, in0=lt_[:], scalar=float(L), in1=x_ap, op0=Op.mult, op1=Op.add)

                # fidx = (L-1 + s - l) mod L   (flipped-mask gather index)
                fidx = small.tile([L, 1], f32, tag="fidx")
                nc.vector.scalar_tensor_tensor(
                    out=fidx[:], in0=s_bc[:], scalar=float(L - 1), in1=iota_p[:],
                    op0=Op.add, op1=Op.subtract)
                mod_l(fidx[:], lo_fix=False, tagp="f")

                # pidx = (L-1 - s + l) mod L   (rolled-psg gather index)
                pidx = small.tile([L, 1], f32, tag="pidx")
                nc.vector.scalar_tensor_tensor(
                    out=pidx[:], in0=s_bc[:], scalar=-1.0, in1=iota_p[:],
                    op0=Op.mult, op1=Op.add)
                nc.vector.tensor_scalar(pidx[:], pidx[:], float(L - 1), None, op0=Op.add)
                mod_l(pidx[:], tagp="p")

                # k = (l - s) mod L ; BOS position is k == 0
                kk = small.tile([L, 1], f32, tag="kk")
                nc.vector.scalar_tensor_tensor(
                    out=kk[:], in0=s_bc[:], scalar=-1.0, in1=iota_p[:],
                    op0=Op.mult, op1=Op.add)
                mod_l(kk[:], hi_fix=False, tagp="k")
                bos = small.tile([L, 1], f32, tag="bos")
                nc.vector.tensor_scalar(bos[:], kk[:], 0.0, None, op0=Op.is_equal)

                # gather mask[fidx] and psg[pidx] (within this batch row)
                fr_i = small.tile([L, 1], i32, tag="fr_i")
                nc.vector.tensor_scalar(fidx[:], fidx[:], float(t * L), None, op0=Op.add)
                nc.vector.tensor_copy(out=fr_i[:], in_=fidx[:])
                mrev = small.tile([L, 1], i32, tag="mrev")
                nc.gpsimd.indirect_dma_start(
                    out=mrev[:], out_offset=None, in_=mask_h[:],
                    in_offset=IndirectOffsetOnAxis(ap=fr_i[:, 0:1], axis=0),
                )
                pr_i = small.tile([L, 1], i32, tag="pr_i")
                nc.vector.tensor_scalar(pidx[:], pidx[:], float(t * L), None, op0=Op.add)
                nc.vector.tensor_copy(out=pr_i[:], in_=pidx[:])
                prot = small.tile([L, 1], i32, tag="prot")
                nc.gpsimd.indirect_dma_start(
                    out=prot[:], out_offset=None, in_=psg_h[:],
                    in_offset=IndirectOffsetOnAxis(ap=pr_i[:, 0:1], axis=0),
                )
                st.update(bos=bos, mrev=mrev, prot=prot)

            def psg_b(t):
                """psg stage 2: combine the gathered rows (long since arrived)
                into trunc/flag; issues the psg embedding gather."""
                st = psg_state[t]
                bos, mrev, prot = st["bos"], st["mrev"], st["prot"]
                # f_rot = 1 - mask[fidx]
                mrev_f = small.tile([L, 1], f32, tag="mrev_f")
                nc.vector.tensor_copy(out=mrev_f[:], in_=mrev[:])
                frot = small.tile([L, 1], f32, tag="frot")
                nc.vector.tensor_scalar(frot[:], mrev_f[:], -1.0, 1.0, op0=Op.mult, op1=Op.add)
                # psg_rot = bos ? 1 : psg[pidx]
                prot_f = small.tile([L, 1], f32, tag="prot_f")
                nc.vector.tensor_copy(out=prot_f[:], in_=prot[:])
                nbos = small.tile([L, 1], f32, tag="nbos")
                nc.vector.tensor_scalar(nbos[:], bos[:], -1.0, 1.0, op0=Op.mult, op1=Op.add)
                nc.vector.tensor_tensor(out=prot_f[:], in0=prot_f[:], in1=nbos[:], op=Op.mult)
                nc.vector.tensor_tensor(out=prot_f[:], in0=prot_f[:], in1=bos[:], op=Op.add)
                # trunc = f_rot * psg_rot
                trunc = small.tile([L, 1], f32, tag="trunc")
                nc.vector.tensor_tensor(out=trunc[:], in0=frot[:], in1=prot_f[:], op=Op.mult)

                # flag = cumsum(trunc != 0) > 0 via triangular matmul
                nz = small.tile([L, 1], f32, tag="nz")
                nc.vector.tensor_scalar(nz[:], trunc[:], 0.0, None, op0=Op.not_equal)
                cnt_ps = psum.tile([L, 1], f32, tag="cnt_ps")
                nc.tensor.matmul(out=cnt_ps[:], lhsT=tri_sb[:], rhs=nz[:], start=True, stop=True)

                trunc_i = small.tile([L, 1], i32, tag="trunc_i")
                nc.vector.tensor_copy(out=trunc_i[:], in_=trunc[:])
                psgemb = emb.tile([L, E], f32, tag="psgemb")
                nc.gpsimd.indirect_dma_start(
                    out=psgemb[:], out_offset=None, in_=wte_h[:],
                    in_offset=IndirectOffsetOnAxis(ap=trunc_i[:, 0:1], axis=0),
                )
                st["cnt_ps"] = cnt_ps
                st["psgemb"] = psgemb

            def psg_c(t):
                """psg stage 3: flag-mask the embeddings and store."""
                st = psg_state[t]
                tok = slice(t * L, (t + 1) * L)
                flag = small.tile([L, 1], f32, tag="flag")
                nc.vector.tensor_scalar(flag[:], st["cnt_ps"][:], 0.0, None, op0=Op.is_gt)
                outp = emb.tile([L, E], f32, tag="outp")
                nc.vector.tensor_scalar(outp[:], st["psgemb"][:], flag[:, 0:1],
                                        None, op0=Op.mult)
                nc.sync.dma_start(out=psgout_h[tok, :], in_=outp[:])

            def partial_tournament(gp, m_all, idx_all):
                """Tournament over spans 0..nsp-2, emitted while the LAST
                span is still streaming: off the terminal critical path."""
                k = nsp - 1
                bidxP = stats.tile([L, nsp], f32, tag="bidxP")
                nc.vector.tensor_tensor(out=bidxP[0:gp, 0:k], in0=bases[0:gp, 0:k],
                                        in1=idx_all[0:gp, 0:k], op=Op.add)
                gmaxP = small.tile([L, 1], f32, tag="gmaxP")
                nc.vector.reduce_max(out=gmaxP[0:gp, :], in_=m_all[0:gp, 0:k], axis=AX.X)
                selP = small.tile([L, nsp], f32, tag="selP")
                nc.vector.scalar_tensor_tensor(
                    out=selP[0:gp, 0:k], in0=m_all[0:gp, 0:k], scalar=gmaxP[0:gp, 0:1],
                    in1=srev[0:gp, 0:k], op0=Op.is_ge, op1=Op.mult)
                cmaxP = small.tile([L, 1], f32, tag="cmaxP")
                nc.vector.reduce_max(out=cmaxP[0:gp, :], in_=selP[0:gp, 0:k], axis=AX.X)
                cstarP = small.tile([L, 1], f32, tag="cstarP")
                nc.vector.tensor_scalar(cstarP[0:gp, :], cmaxP[0:gp, :], -1.0, float(nsp),
                                        op0=Op.mult, op1=Op.add)
                junkP = small.tile([L, nsp], f32, tag="junkP")
                nc.vector.scalar_tensor_tensor(
                    out=junkP[0:gp, 0:k], in0=iotas[0:gp, 0:k], scalar=cstarP[0:gp, 0:1],
                    in1=bidxP[0:gp, 0:k], op0=Op.is_equal, op1=Op.mult)
                hotP = small.tile([L, 1], f32, tag="hotP")
                nc.vector.reduce_max(out=hotP[0:gp, :], in_=junkP[0:gp, 0:k], axis=AX.X)
                return gmaxP, hotP

            def stream_group(g0_, gp, mode, interleave=None):
                """DMA-bound pass over the vocab for rows [g0_, g0_+gp):
                per span, load logits, accumulate gumbel in the DMA
                datapath, then track span maxes (and, in maxidx mode, the
                first within-span argmax position).  `interleave` is the
                prior group's deferred rescan finish, emitted a few chunks
                in -- by then its re-read data has arrived, so the max_index
                never stalls the in-order DVE queue."""
                rows = slice(g0_, g0_ + gp)
                idx_all = None
                if mode == "maxidx":
                    m_all = stats.tile([L, nsp], f32, tag="m_all" + mode)
                    idx_all = stats.tile([L, nsp], f32, tag="idx_all")
                    spans = spans_mi
                else:
                    m_all = stats.tile([L, nfine], f32, tag="m_all" + mode)
                    spans = [(c * ch, ch) for c in range(nch)]
                pool = lpool if g0_ == 0 else lpool1
                partial = None
                for c, (lo, w) in enumerate(spans):
                    for fn in (interleave or {}).get(c, []):
                        fn()
                    if mode == "maxidx" and c == len(spans) - 1:
                        partial = partial_tournament(gp, m_all, idx_all)
                    lt = pool.tile([L, ch], f32, tag="lt")
                    ldeng = nc.scalar if c % 2 else nc.sync
                    ldeng.dma_start(out=lt[0:gp, 0:w], in_=lg_h[rows, lo:lo + w])
                    nc.gpsimd.dma_start(
                        out=lt[0:gp, 0:w], in_=gm_h[rows, lo:lo + w], accum_op=Op.add)
                    if mode == "maxidx":
                        mx8 = small.tile([L, 8], f32, tag="mx8")
                        nc.vector.max(out=mx8[0:gp, :], in_=lt[0:gp, 0:w])
                        nc.vector.tensor_copy(out=m_all[0:gp, c:c + 1], in_=mx8[0:gp, 0:1])
                        mi8 = small.tile([L, 8], u32, tag="mi8s")
                        nc.vector.max_index(out=mi8[0:gp, :], in_max=mx8[0:gp, :],
                                            in_values=lt[0:gp, 0:w])
                        nc.vector.tensor_copy(out=idx_all[0:gp, c:c + 1], in_=mi8[0:gp, 0:1])
                    else:
                        # chunk maxes at sub-chunk (sw-wide) granularity
                        lt3 = lt[0:gp, 0:w].rearrange("p (a b) -> p a b", b=sw)
                        nc.vector.reduce_max(
                            out=m_all[0:gp, c * SUB:(c + 1) * SUB], in_=lt3, axis=AX.X)
                return m_all, idx_all, partial

            def tournament(g0_, gp, m_all, rev, nslots):
                """global max + LOWEST slot attaining it (argmax tie rule)."""
                gmax = small.tile([L, 1], f32, tag="gmax")
                nc.vector.reduce_max(out=gmax[0:gp, :], in_=m_all[0:gp, :], axis=AX.X)
                sel = small.tile([L, nslots], f32, tag="sel" + str(nslots))
                nc.vector.scalar_tensor_tensor(
                    out=sel[0:gp, :], in0=m_all[0:gp, :], scalar=gmax[0:gp, 0:1],
                    in1=rev[0:gp, :], op0=Op.is_ge, op1=Op.mult)
                cmax = small.tile([L, 1], f32, tag="cmax")
                nc.vector.reduce_max(out=cmax[0:gp, :], in_=sel[0:gp, :], axis=AX.X)
                cstar = small.tile([L, 1], f32, tag="cstar")
                nc.vector.tensor_scalar(cstar[0:gp, :], cmax[0:gp, :], -1.0, float(nslots),
                                        op0=Op.mult, op1=Op.add)
                return gmax, cstar

            def finish_group(g0_, gp, hot_f, split=False):
                """hot -> W_pre gather -> tok_out rows.  split pipelines the
                gather/store pair in two row halves (shorter terminal chain)."""
                hot_i = small.tile([L, 1], i32, tag="hot_i")
                nc.vector.tensor_copy(out=hot_i[0:gp, :], in_=hot_f[0:gp, :])
                if gp == 1:
                    # single-partition indirect DMAs are rejected; pad with a
                    # zeroed second index and gather one junk row (never read)
                    nc.vector.memset(hot_i[1:2, :], 0)
                halves = [(0, gp)] if not split or gp < 32 else [
                    (0, gp // 2), (gp // 2, gp)]
                tokemb = emb.tile([L, E], f32, tag="tokemb")
                for (a, b) in halves:
                    bi = 2 if (b - a == 1 and a == 0) else b
                    nc.gpsimd.indirect_dma_start(
                        out=tokemb[a:bi, :], out_offset=None, in_=wpre_h[:],
                        in_offset=IndirectOffsetOnAxis(ap=hot_i[a:bi, 0:1], axis=0))
                steng = {"swdge": nc.gpsimd, "act": nc.scalar}.get(
                    d.get("STORE_ENG", "sp"), nc.sync)
                for (a, b) in halves:
                    steng.dma_start(out=tokout_h[g0_ + a:g0_ + b, :], in_=tokemb[a:b, :])

            def tail_rescan_start(g0_, gp, m_all):
                """Tournament (all data ready -> no DVE stalls) + issue the
                winning-sub-chunk re-read. The dependent max_index is deferred
                (tail_rescan_end) so its DMA wait never blocks the in-order
                DVE queue while the next group streams."""
                gmax, sstar = tournament(g0_, gp, m_all, frev, nfine)
                # fine-sliced row index for the re-read: (row + g0)*nfine + sstar
                ridx_f = small.tile([L, 1], f32, tag="ridx_f")
                nc.vector.tensor_tensor(out=ridx_f[0:gp, :], in0=rowb[g0_][0:gp, :],
                                        in1=sstar[0:gp, :], op=Op.add)
                ridx_i = small.tile([L, 1], i32, tag="ridx_i")
                nc.vector.tensor_copy(out=ridx_i[0:gp, :], in_=ridx_f[0:gp, :])
                gmax8 = small.tile([L, 8], f32, tag="gmax8")
                nc.vector.tensor_copy(out=gmax8[0:gp, :],
                                      in_=gmax[0:gp, 0:1].to_broadcast([gp, 8]))

                # single-partition indirect DMAs are rejected; pad the index
                # tile with a zeroed row and gather one junk row (never read)
                gpi = 2 if gp == 1 else gp
                if gpi != gp:
                    nc.vector.memset(ridx_i[gp:gpi, :], 0)

                # re-read ONLY the winning sub-chunk (logits + CCE-add gumbel);
                # bit-matches the stream pass, so gmax is found exactly
                rs = emb.tile([L, sw], f32, tag="rs")
                nc.gpsimd.indirect_dma_start(
                    out=rs[0:gpi, :], out_offset=None, in_=lg_v,
                    in_offset=IndirectOffsetOnAxis(ap=ridx_i[0:gpi, 0:1], axis=0))
                nc.gpsimd.indirect_dma_start(
                    out=rs[0:gpi, :], out_offset=None, in_=gm_v,
                    in_offset=IndirectOffsetOnAxis(ap=ridx_i[0:gpi, 0:1], axis=0),
                    compute_op=Op.add)
                return sstar, gmax8, rs

            def tail_rescan_end(g0_, gp, state):
                sstar, gmax8, rs = state
                mi8 = small.tile([L, 8], u32, tag="mi8")
                nc.vector.max_index(out=mi8[0:gp, :], in_max=gmax8[0:gp, :],
                                    in_values=rs[0:gp, :])
                mi_f = small.tile([L, 1], f32, tag="mi_f")
                nc.vector.tensor_copy(out=mi_f[0:gp, :], in_=mi8[0:gp, 0:1])
                hot_f = small.tile([L, 1], f32, tag="hot_f")
                nc.vector.scalar_tensor_tensor(
                    out=hot_f[0:gp, :], in0=sstar[0:gp, :], scalar=float(sw),
                    in1=mi_f[0:gp, :], op0=Op.mult, op1=Op.add)
                finish_group(g0_, gp, hot_f)

            def tail_maxidx(g0_, gp, m_all, idx_all, partial):
                # combine the pre-computed partial tournament with the last
                # span: the last span wins only on STRICTLY greater (argmax
                # takes the first occurrence, i.e. the earlier span on ties)
                gmaxP, hotP = partial
                lo_last = spans_mi[-1][0]
                hotL = small.tile([L, 1], f32, tag="hotL")
                nc.vector.tensor_scalar(hotL[0:gp, :], idx_all[0:gp, nsp - 1:nsp],
                                        1.0, float(lo_last), op0=Op.mult, op1=Op.add)
                gt = small.tile([L, 1], f32, tag="gtL")
                nc.vector.tensor_tensor(out=gt[0:gp, :], in0=m_all[0:gp, nsp - 1:nsp],
                                        in1=gmaxP[0:gp, :], op=Op.is_gt)
                dd = small.tile([L, 1], f32, tag="ddL")
                nc.vector.tensor_tensor(out=dd[0:gp, :], in0=hotL[0:gp, :],
                                        in1=hotP[0:gp, :], op=Op.subtract)
                hot_f = small.tile([L, 1], f32, tag="hot_f")
                nc.vector.tensor_tensor(out=hot_f[0:gp, :], in0=gt[0:gp, :],
                                        in1=dd[0:gp, :], op=Op.mult)
                nc.vector.tensor_tensor(out=hot_f[0:gp, :], in0=hot_f[0:gp, :],
                                        in1=hotP[0:gp, :], op=Op.add)
                finish_group(g0_, gp, hot_f, split=d.get("WSPLIT", False))

            # psg stage 1a first: only the mask load + cold-PE matmul issue,
            # so the DVE queue stays clear for the first chunk maxes
            for t in range(2):
                psg_a(t)
            pending = {d.get("ILA", 4): [lambda: [psg_a2(t) for t in range(2)]]}
            for gi, (g0_, gp, mode) in enumerate(groups):
                last = gi == len(groups) - 1
                m_all, idx_all, partial = stream_group(g0_, gp, mode,
                                                        interleave=pending)
                pending = {}
                if not last:
                    # non-terminal group: tournament + issue the re-read now;
                    # the dependent finish and psg stages 2/3 slot a few
                    # chunks into the next group's stream (data ready by then)
                    state = tail_rescan_start(g0_, gp, m_all)
                    for t in range(2):
                        psg_b(t)
                    pending = {
                        d.get("ILB", 13): [
                            lambda g0=g0_, gp_=gp, st=state: tail_rescan_end(g0, gp_, st)],
                        d.get("ILC", 8): [lambda: [psg_c(t) for t in range(2)]],
                    }
                else:
                    if len(groups) == 1:
                        for t in range(2):
                            psg_b(t)
                    if mode == "maxidx":
                        tail_maxidx(g0_, gp, m_all, idx_all, partial)
                    else:
                        tail_rescan_end(g0_, gp, tail_rescan_start(g0_, gp, m_all))
                    if len(groups) == 1:
                        for t in range(2):
                            psg_c(t)

    return nc


_BUILD_CACHE = {}


def _get_module_v2(cap, dims_key=None, dims=None):
    key = ("v2", cap, dims_key)
    if key not in _BUILD_CACHE:
        import concourse.bacc as bacc

        nc = bacc.Bacc("TRN2", target_bir_lowering=False, debug=False)
        _build_v2(nc, cap, dims)
        nc.compile()
        _BUILD_CACHE[key] = nc
    return _BUILD_CACHE[key]


_ROWMAP_CACHE = {}


def _nearest_maps():
    """Replicate the reference's f32 grid_sample-nearest index maps with jnp
    on the same backend the reference runs on (bit-exact by construction)."""
    if "maps" not in _ROWMAP_CACHE:
        import jax.numpy as jnp

        def nearest(size):
            lin = jnp.linspace(-1.0, 1.0, size)
            ix = ((lin + 1.0) * size - 1.0) / 2.0
            return np.asarray(jnp.clip(jnp.round(ix), 0, size - 1).astype(jnp.int32))

        _ROWMAP_CACHE["maps"] = (nearest(V), nearest(E))
    return _ROWMAP_CACHE["maps"]


_TRI = None

# test/dev hooks: set TRACE=True before calling kernel() to capture an NTFF
# profile; the BassKernelResults of the last run is stored in LAST_RESULT.
TRACE = False
LAST_RESULT = None
LAST_MODULE = None


def kernel(logits, rwrt_attention_mask, psg_input_ids, word_embeddings, gumbel_noise):
    from concourse.bass_utils import run_bass_kernel_spmd

    global _TRI, LAST_RESULT, LAST_MODULE
    logits = np.ascontiguousarray(np.asarray(logits, dtype=np.float32))
    gumbel = np.ascontiguousarray(np.asarray(gumbel_noise, dtype=np.float32))
    mask = np.ascontiguousarray(np.asarray(rwrt_attention_mask, dtype=np.int32))
    psg = np.ascontiguousarray(np.asarray(psg_input_ids, dtype=np.int32))
    wte = np.ascontiguousarray(np.asarray(word_embeddings, dtype=np.float32))

    rowmap, colmap = _nearest_maps()
    # token branch reads W[rowmap][:, colmap]; constant-fold both static maps
    wpre = np.ascontiguousarray(wte[rowmap][:, colmap])
    if _TRI is None:
        _TRI = np.ascontiguousarray(np.triu(np.ones((L, L), dtype=np.float32)))

    # ---- sharding: compact the active (mask==1) tokens across cores ----
    flat_mask = mask.reshape(B * L) != 0
    ids = np.nonzero(flat_mask)[0].astype(np.int64)    # batch-major order
    n_act = int(ids.size)
    cap = max(8, -(-n_act // N_CORES))                 # tokens per core
    cap += cap & 1     # even row counts only: an odd-cap module (odd-row
                       # groups / odd-row tok_out) hit NRT_EXEC_UNIT_
                       # UNRECOVERABLE on device; one pad slot is free
    pad_id = int(ids[-1]) if n_act else 0
    ids_p = np.full(N_CORES * cap, pad_id, dtype=np.int64)
    ids_p[:n_act] = ids
    lg_flat = logits.reshape(B * L, V)
    gm_flat = gumbel.reshape(B * L, V)

    nc = _get_module_v2(cap)
    LAST_MODULE = nc

    in_maps = []
    for m in range(N_CORES):
        sl = ids_p[m * cap:(m + 1) * cap]
        im = {
            "lg": np.ascontiguousarray(lg_flat[sl]),
            "gm": np.ascontiguousarray(gm_flat[sl]),
            "mask": mask[2 * m:2 * m + 2].reshape(2 * L, 1),
            "psg": psg[2 * m:2 * m + 2].reshape(2 * L, 1),
            "wte": wte,
            "wpre": wpre,
            "tri": _TRI,
        }
        in_maps.append(im)

    try:
        LAST_RESULT = run_bass_kernel_spmd(nc, in_maps, list(range(N_CORES)), trace=TRACE)
    except Exception:
        # the axon-relayed device occasionally reports a transient
        # NRT_EXEC_UNIT_UNRECOVERABLE; re-runs (with backoff) recover it
        import time as _time

        for backoff in (2.0, 15.0):
            _time.sleep(backoff)
            try:
                LAST_RESULT = run_bass_kernel_spmd(
                    nc, in_maps, list(range(N_CORES)), trace=TRACE)
                break
            except Exception:
                if backoff == 15.0:
                    raise
    res = LAST_RESULT.results

    # ---- unshard: psg partial everywhere + scatter-add the token rows ----
    out = np.concatenate(
        [res[m]["psg_out"].reshape(2, L, E) for m in range(N_CORES)], axis=0)
    if n_act:
        tok = np.concatenate(
            [res[m]["tok_out"].reshape(cap, E) for m in range(N_CORES)], axis=0)
        flat = out.reshape(B * L, E)
        flat[ids] += tok[:n_act]
    return out


# revision 42
# speedup vs baseline: 1.0062x; 1.0002x over previous
"""Trainium2 Bass kernel: gumbel-softmax-argmax embedding lookup (end-to-end).

Reference math (nn_End2End_49495203119139):
    hot  = argmax_V(softmax((logits + gumbel)/tau))       == argmax_V(logits+gumbel)
    row  = grid_sample-nearest index map of hot            == ROWMAP[hot]  (LUT)
    tok_emb = W[row][:, col_map]   with col_map == arange(E)  (verified at runtime)
    inputs_embeds = tok_emb * mask
    psg_roll = roll(psg_ids, 1, axis=1); psg_roll[:,0] = 1
    extr  = (1 - mask[:, ::-1]) * psg_roll
    trunc = rotate_right(extr, shifts) with shifts = mask.sum(-1)   (per row)
    flag  = cumsum(trunc != 0, -1) > 0
    out   = inputs_embeds + where(flag, W[trunc], 0)

Key observation: inputs_embeds is multiplied by the attention mask, so the
argmax over the 32128-wide vocab -- the part that forces streaming the
526 MB of logits+gumbel -- is only needed for tokens with mask == 1.
The mask is an input, so sharding can compact: the host flattens the
active (b, l) positions (N of B*L), pads to C = ceil(N/8) per core, and
ships each core a compacted [C, V] slice of logits and gumbel.  Each core
streams only its C rows (vs B*L/8 = 256 for dense data-parallel).  The
passage branch (cheap) stays on the owner core of each batch row; the two
partial outputs (psg_out for all positions, tok_out rows for the active
tokens) are combined by the host during the unshard step -- an exact f32
add of the same values the dense kernel would have added on-device.

Per-core device plan (memory-bound part = streaming the compacted
logits+gumbel, 2*C*V*4 bytes ~= 45.8 MB for C=178, ~127 us HBM floor at
360 GB/s per core).  Rows go in two partition groups, (C-128, 128); per
group and per vocab span: HWDGE-load the logits span (alternating SP/ACT
DMA queues), add the gumbel span with one SWDGE inline-accumulate DMA
(CCE add; descriptors must stay <= 2048 elements), then find the span
max on DVE.  The argmax strategy differs per group because DVE is the
scarce engine after compaction (max+max_index of every span would put
~145 us on DVE vs the ~135 us DMA floor):
  - FIRST group (C-128 rows): max-only streaming -- one reduce_max per
    span at 251-element sub-span granularity.  A short tournament picks
    the LOWEST sub-span attaining the global max (argmax tie rule), then
    that sub-span alone is re-read (indirect row gather + CCE-add) and
    ONE max_index against the broadcast global max finds the in-span
    position -- exact because the CCE re-add bit-matches the stream
    pass.  Its tail hides under the second group's stream; the max_index
    is deferred a few spans into it so the re-read's DMA latency never
    blocks the in-order DVE queue.
  - LAST group (128 rows; carries the terminal tail): per-span max +
    max_index inline (affordable: its 5.7 us/span DMA outpaces the
    4.4 us/span DVE cost), so after the final span only the tournament
    and the embedding gather remain.  The vocab end is tapered into
    shrinking spans so the final span's DVE work is short.
  - hot -> token embedding via ONE indirect gather from a host-built
    table W_pre = W[rowmap][:, colmap] (constant-folds the grid_sample
    LUTs, removing a dependent gather from the critical tail).
  - the passage branch is pure index arithmetic on [128,1] tiles: the
    reverse/roll/rotate are folded into gather indices modulo L, the
    mask-sum and cumsum are 0/1 matmuls against ones/triangular matrices
    (exact in any PE precision).  psg_out = flag * W[trunc] is stored for
    every position.  It is emitted in three stages (index math + row
    gathers up front, combine after the first group's stream, flag+store
    mid second stream) so its cross-engine waits never stall the DVE
    queue ahead of stream work.
"""

import numpy as np

B = 16
L = 128
V = 32128
E = 768
N_CORES = 8
CH = 2008                     # vocab chunk (free dim) per streamed tile;
                              # <= 2048 elements so a gumbel chunk is ONE
                              # CCE-add DMA descriptor per partition row
NCH = V // CH                 # 16 chunks


def _build_v2(nc_mod, cap, dims=None):
    """Per-core module for the compacted layout. cap = tokens per core."""
    import concourse.tile as tile
    from concourse import bass, mybir
    from concourse.bass import IndirectOffsetOnAxis

    d = dims or {}
    ch = d.get("CH", CH)
    nch = V // ch
    lbufs = d.get("LBUFS", 10)
    # rescan-mode groups track chunk maxes at sub-chunk granularity so the
    # winning-chunk re-read (and its max_index) touch only SW elements
    SUB = ch // 251
    sw = 251                      # V == nch*SUB*sw exactly
    nfine = nch * SUB             # 128 fine slots
    # maxidx-mode groups taper the end of the vocab so the last spans'
    # (max + max_index) DVE work after the final CCE-add is short
    taper = d.get("TAPER", (1255, 1255, 1255, 251) if ch == 2008 else
                  (502, 502, 502, 502))
    nfull = nch - sum(taper) // ch
    assert sum(taper) % ch == 0
    spans_mi = [(c * ch, ch) for c in range(nfull)]
    tail_lo = nfull * ch
    for w_ in taper:
        spans_mi.append((tail_lo, w_))
        tail_lo += w_
    assert tail_lo == V
    nsp = len(spans_mi)

    nc = nc_mod
    f32 = mybir.dt.float32
    i32 = mybir.dt.int32
    u32 = mybir.dt.uint32
    Op = mybir.AluOpType
    AX = mybir.AxisListType

    # partition groups covering the cap rows.  The LAST group carries the
    # terminal tail, so it uses baseline per-chunk max_index (short tail, no
    # re-read) -- affordable only when its DMA per chunk (rows*ch*8B/360GBps)
    # outpaces the ~4.4us/chunk of DVE work, i.e. rows >= ~100.  Small groups
    # use max-only streaming with a winning-chunk re-read; their longer tail
    # hides under the next group's stream.
    groups = []  # (row0, rows, mode)
    g1mode = d.get("G1MODE", "maxidx")
    if cap > 128:
        g1rows = min(d.get("G1ROWS", 128), cap)
        g0rows = cap - g1rows
        if g0rows < 48:
            # very small streaming groups crashed the device (validated down
            # to 48 rows); rebalance and drop to rescan mode if the terminal
            # group gets too small for inline max_index to keep DVE fed
            g0rows = 48
            g1rows = cap - g0rows
        mode1 = g1mode if g1rows >= 100 else "rescan"
        groups.append((0, g0rows, "rescan"))
        groups.append((g0rows, g1rows, mode1))
    else:
        groups.append((0, cap, g1mode if cap >= 128 else "rescan"))

    lg_h = nc.dram_tensor("lg", [cap, V], f32, kind="ExternalInput")
    gm_h = nc.dram_tensor("gm", [cap, V], f32, kind="ExternalInput")
    mask_h = nc.dram_tensor("mask", [2 * L, 1], i32, kind="ExternalInput")
    psg_h = nc.dram_tensor("psg", [2 * L, 1], i32, kind="ExternalInput")
    wte_h = nc.dram_tensor("wte", [V, E], f32, kind="ExternalInput")
    wpre_h = nc.dram_tensor("wpre", [V, E], f32, kind="ExternalInput")
    tri_h = nc.dram_tensor("tri", [L, L], f32, kind="ExternalInput")
    psgout_h = nc.dram_tensor("psg_out", [2 * L, E], f32, kind="ExternalOutput")
    tokout_h = nc.dram_tensor("tok_out", [cap, E], f32, kind="ExternalOutput")

    # fine-sliced row views for the data-dependent winning-slot re-read
    lg_v = lg_h[:].rearrange("r (c w) -> (r c) w", w=sw)
    gm_v = gm_h[:].rearrange("r (c w) -> (r c) w", w=sw)

    with tile.TileContext(nc) as tc:
        with (
            tc.tile_pool(name="lpool", bufs=lbufs) as lpool,
            tc.tile_pool(name="lpool1", bufs=d.get("LBUFS1", 5)) as lpool1,
            tc.tile_pool(name="stats", bufs=2) as stats,
            tc.tile_pool(name="small", bufs=2) as small,
            tc.tile_pool(name="emb", bufs=2) as emb,
            tc.tile_pool(name="consts", bufs=1) as consts,
            tc.tile_pool(name="psum", bufs=2, space="PSUM") as psum,
        ):
            # ---- per-core constants (built once) ----
            ones_mat = consts.tile([L, L], f32)
            nc.vector.memset(ones_mat[:], 1.0)
            iota_p_i = consts.tile([L, 1], i32)
            nc.gpsimd.iota(iota_p_i[:], pattern=[[1, 1]], base=0, channel_multiplier=1)
            iota_p = consts.tile([L, 1], f32)
            nc.vector.tensor_copy(out=iota_p[:], in_=iota_p_i[:])

            # tri[i,j] = (j >= i) built on device: one less DMA
            tri_sb = consts.tile([L, L], f32)
            if d.get("TRI_DEV", True):
                trij_i = consts.tile([L, L], i32)
                nc.gpsimd.iota(trij_i[:], pattern=[[1, L]], base=0, channel_multiplier=0)
                trij = consts.tile([L, L], f32)
                nc.vector.tensor_copy(out=trij[:], in_=trij_i[:])
                nc.vector.tensor_scalar(tri_sb[:], trij[:], iota_p[:, 0:1], None, op0=Op.is_ge)
            else:
                nc.gpsimd.dma_start(out=tri_sb[:], in_=tri_h[:])

            need_mi = any(m == "maxidx" for (_, _, m) in groups)
            need_rs = any(m == "rescan" for (_, _, m) in groups)
            if need_mi:
                iotas_i = consts.tile([L, nsp], i32)
                nc.gpsimd.iota(iotas_i[:], pattern=[[1, nsp]], base=0, channel_multiplier=0)
                iotas = consts.tile([L, nsp], f32)
                nc.vector.tensor_copy(out=iotas[:], in_=iotas_i[:])
                # srev[c] = nsp - c (used to pick the LOWEST span attaining the max)
                srev = consts.tile([L, nsp], f32)
                nc.vector.tensor_scalar(srev[:], iotas[:], -1.0, float(nsp), op0=Op.mult, op1=Op.add)
                # per-span start offsets (hot = bases[c*] + within-span index)
                bases = consts.tile([L, nsp], f32)
                nc.vector.tensor_scalar(bases[:], iotas[:], float(ch), None, op0=Op.mult)
                for ci, (lo_c, _w) in enumerate(spans_mi):
                    if lo_c != ci * ch:
                        nc.vector.memset(bases[:, ci:ci + 1], float(lo_c))
            if need_rs:
                iotaf_i = consts.tile([L, nfine], i32)
                nc.gpsimd.iota(iotaf_i[:], pattern=[[1, nfine]], base=0, channel_multiplier=0)
                iotaf = consts.tile([L, nfine], f32)
                nc.vector.tensor_copy(out=iotaf[:], in_=iotaf_i[:])
                frev = consts.tile([L, nfine], f32)
                nc.vector.tensor_scalar(frev[:], iotaf[:], -1.0, float(nfine), op0=Op.mult, op1=Op.add)
            # per-partition fine-sliced row base: (row + group offset) * nfine
            rowb = {}
            for (g0_, _gp, mode) in groups:
                if mode != "rescan":
                    continue
                rb = consts.tile([L, 1], f32)
                nc.vector.tensor_scalar(rb[:], iota_p[:], float(nfine), float(g0_ * nfine),
                                        op0=Op.mult, op1=Op.add)
                rowb[g0_] = rb

            psg_state = {}

            def psg_a(t):
                """psg stage 1: mask load, length, gather indices; issues the
                mask/psg row gathers.  All DVE ops here depend only on the
                mask load + ones matmul, so they run during kernel startup."""
                tok = slice(t * L, (t + 1) * L)
                mask_i = small.tile([L, 1], i32, tag="mask_i")
                nc.scalar.dma_start(out=mask_i[:], in_=mask_h[tok, :])
                mask_f = small.tile([L, 1], f32, tag="mask_f")
                nc.vector.tensor_copy(out=mask_f[:], in_=mask_i[:])

                # s (broadcast to all partitions) = sum(mask) via ones matmul
                s_ps = psum.tile([L, 1], f32, tag="s_ps")
                nc.tensor.matmul(out=s_ps[:], lhsT=ones_mat[:], rhs=mask_f[:], start=True, stop=True)
                psg_state[t] = dict(s_ps=s_ps)

            def psg_a2(t):
                """psg stage 1b: index arithmetic (the cold-PE matmul from
                stage 1a is long done, so nothing here stalls the DVE queue);
                issues the mask/psg row gathers."""
                st = psg_state[t]
                s_ps = st["s_ps"]
                s_bc = small.tile([L, 1], f32, tag="s_bc")
                nc.vector.tensor_copy(out=s_bc[:], in_=s_ps[:])

                def mod_l(x_ap, lo_fix=True, hi_fix=True, tagp=""):
                    # x <- x mod L for x in (-L, 2L)
                    if hi_fix:
                        ge = small.tile([L, 1], f32, tag="ge" + tagp)
                        nc.vector.tensor_scalar(ge[:], x_ap, float(L), None, op0=Op.is_ge)
                        nc.vector.scalar_tensor_tensor(
                            out=x_ap, in0=ge[:], scalar=-float(L), in1=x_ap, op0=Op.mult, op1=Op.add)
                    if lo_fix:
                        lt_ = small.tile([L, 1], f32, tag="lt" + tagp)
                        nc.vector.tensor_scalar(lt_[:], x_ap, 0.0, None, op0=Op.is_lt)
                        nc.vector.scalar_tensor_tensor(
                            out=x_ap, in0=lt_[:], scalar=float(L), in1=x_ap, op0=Op.mult, op1=Op.add)

                # fidx = (L-1 + s - l) mod L   (flipped-mask gather index)
                fidx = small.tile([L, 1], f32, tag="fidx")
                nc.vector.scalar_tensor_tensor(
                    out=fidx[:], in0=s_bc[:], scalar=float(L - 1), in1=iota_p[:],
                    op0=Op.add, op1=Op.subtract)
                mod_l(fidx[:], lo_fix=False, tagp="f")

                # pidx = (L-1 - s + l) mod L   (rolled-psg gather index)
                pidx = small.tile([L, 1], f32, tag="pidx")
                nc.vector.scalar_tensor_tensor(
                    out=pidx[:], in0=s_bc[:], scalar=-1.0, in1=iota_p[:],
                    op0=Op.mult, op1=Op.add)
                nc.vector.tensor_scalar(pidx[:], pidx[:], float(L - 1), None, op0=Op.add)
                mod_l(pidx[:], tagp="p")

                # k = (l - s) mod L ; BOS position is k == 0
                kk = small.tile([L, 1], f32, tag="kk")
                nc.vector.scalar_tensor_tensor(
                    out=kk[:], in0=s_bc[:], scalar=-1.0, in1=iota_p[:],
                    op0=Op.mult, op1=Op.add)
                mod_l(kk[:], hi_fix=False, tagp="k")
                bos = small.tile([L, 1], f32, tag="bos")
                nc.vector.tensor_scalar(bos[:], kk[:], 0.0, None, op0=Op.is_equal)

                # gather mask[fidx] and psg[pidx] (within this batch row)
                fr_i = small.tile([L, 1], i32, tag="fr_i")
                nc.vector.tensor_scalar(fidx[:], fidx[:], float(t * L), None, op0=Op.add)
                nc.vector.tensor_copy(out=fr_i[:], in_=fidx[:])
                mrev = small.tile([L, 1], i32, tag="mrev")
                nc.gpsimd.indirect_dma_start(
                    out=mrev[:], out_offset=None, in_=mask_h[:],
                    in_offset=IndirectOffsetOnAxis(ap=fr_i[:, 0:1], axis=0),
                )
                pr_i = small.tile([L, 1], i32, tag="pr_i")
                nc.vector.tensor_scalar(pidx[:], pidx[:], float(t * L), None, op0=Op.add)
                nc.vector.tensor_copy(out=pr_i[:], in_=pidx[:])
                prot = small.tile([L, 1], i32, tag="prot")
                nc.gpsimd.indirect_dma_start(
                    out=prot[:], out_offset=None, in_=psg_h[:],
                    in_offset=IndirectOffsetOnAxis(ap=pr_i[:, 0:1], axis=0),
                )
                st.update(bos=bos, mrev=mrev, prot=prot)

            def psg_b(t):
                """psg stage 2: combine the gathered rows (long since arrived)
                into trunc/flag; issues the psg embedding gather."""
                st = psg_state[t]
                bos, mrev, prot = st["bos"], st["mrev"], st["prot"]
                # f_rot = 1 - mask[fidx]
                mrev_f = small.tile([L, 1], f32, tag="mrev_f")
                nc.vector.tensor_copy(out=mrev_f[:], in_=mrev[:])
                frot = small.tile([L, 1], f32, tag="frot")
                nc.vector.tensor_scalar(frot[:], mrev_f[:], -1.0, 1.0, op0=Op.mult, op1=Op.add)
                # psg_rot = bos ? 1 : psg[pidx]
                prot_f = small.tile([L, 1], f32, tag="prot_f")
                nc.vector.tensor_copy(out=prot_f[:], in_=prot[:])
                nbos = small.tile([L, 1], f32, tag="nbos")
                nc.vector.tensor_scalar(nbos[:], bos[:], -1.0, 1.0, op0=Op.mult, op1=Op.add)
                nc.vector.tensor_tensor(out=prot_f[:], in0=prot_f[:], in1=nbos[:], op=Op.mult)
                nc.vector.tensor_tensor(out=prot_f[:], in0=prot_f[:], in1=bos[:], op=Op.add)
                # trunc = f_rot * psg_rot
                trunc = small.tile([L, 1], f32, tag="trunc")
                nc.vector.tensor_tensor(out=trunc[:], in0=frot[:], in1=prot_f[:], op=Op.mult)

                # flag = cumsum(trunc != 0) > 0 via triangular matmul
                nz = small.tile([L, 1], f32, tag="nz")
                nc.vector.tensor_scalar(nz[:], trunc[:], 0.0, None, op0=Op.not_equal)
                cnt_ps = psum.tile([L, 1], f32, tag="cnt_ps")
                nc.tensor.matmul(out=cnt_ps[:], lhsT=tri_sb[:], rhs=nz[:], start=True, stop=True)

                trunc_i = small.tile([L, 1], i32, tag="trunc_i")
                nc.vector.tensor_copy(out=trunc_i[:], in_=trunc[:])
                psgemb = emb.tile([L, E], f32, tag="psgemb")
                nc.gpsimd.indirect_dma_start(
                    out=psgemb[:], out_offset=None, in_=wte_h[:],
                    in_offset=IndirectOffsetOnAxis(ap=trunc_i[:, 0:1], axis=0),
                )
                st["cnt_ps"] = cnt_ps
                st["psgemb"] = psgemb

            def psg_c(t):
                """psg stage 3: flag-mask the embeddings and store."""
                st = psg_state[t]
                tok = slice(t * L, (t + 1) * L)
                flag = small.tile([L, 1], f32, tag="flag")
                nc.vector.tensor_scalar(flag[:], st["cnt_ps"][:], 0.0, None, op0=Op.is_gt)
                outp = emb.tile([L, E], f32, tag="outp")
                nc.vector.tensor_scalar(outp[:], st["psgemb"][:], flag[:, 0:1],
                                        None, op0=Op.mult)
                nc.sync.dma_start(out=psgout_h[tok, :], in_=outp[:])

            def partial_tournament(gp, m_all, idx_all):
                """Tournament over spans 0..nsp-2, emitted while the LAST
                span is still streaming: off the terminal critical path."""
                k = nsp - 1
                bidxP = stats.tile([L, nsp], f32, tag="bidxP")
                nc.vector.tensor_tensor(out=bidxP[0:gp, 0:k], in0=bases[0:gp, 0:k],
                                        in1=idx_all[0:gp, 0:k], op=Op.add)
                gmaxP = small.tile([L, 1], f32, tag="gmaxP")
                nc.vector.reduce_max(out=gmaxP[0:gp, :], in_=m_all[0:gp, 0:k], axis=AX.X)
                selP = small.tile([L, nsp], f32, tag="selP")
                nc.vector.scalar_tensor_tensor(
                    out=selP[0:gp, 0:k], in0=m_all[0:gp, 0:k], scalar=gmaxP[0:gp, 0:1],
                    in1=srev[0:gp, 0:k], op0=Op.is_ge, op1=Op.mult)
                cmaxP = small.tile([L, 1], f32, tag="cmaxP")
                nc.vector.reduce_max(out=cmaxP[0:gp, :], in_=selP[0:gp, 0:k], axis=AX.X)
                cstarP = small.tile([L, 1], f32, tag="cstarP")
                nc.vector.tensor_scalar(cstarP[0:gp, :], cmaxP[0:gp, :], -1.0, float(nsp),
                                        op0=Op.mult, op1=Op.add)
                junkP = small.tile([L, nsp], f32, tag="junkP")
                nc.vector.scalar_tensor_tensor(
                    out=junkP[0:gp, 0:k], in0=iotas[0:gp, 0:k], scalar=cstarP[0:gp, 0:1],
                    in1=bidxP[0:gp, 0:k], op0=Op.is_equal, op1=Op.mult)
                hotP = small.tile([L, 1], f32, tag="hotP")
                nc.vector.reduce_max(out=hotP[0:gp, :], in_=junkP[0:gp, 0:k], axis=AX.X)
                return gmaxP, hotP

            def stream_group(g0_, gp, mode, interleave=None):
                """DMA-bound pass over the vocab for rows [g0_, g0_+gp):
                per span, load logits, accumulate gumbel in the DMA
                datapath, then track span maxes (and, in maxidx mode, the
                first within-span argmax position).  `interleave` is the
                prior group's deferred rescan finish, emitted a few chunks
                in -- by then its re-read data has arrived, so the max_index
                never stalls the in-order DVE queue."""
                rows = slice(g0_, g0_ + gp)
                idx_all = None
                if mode == "maxidx":
                    m_all = stats.tile([L, nsp], f32, tag="m_all" + mode)
                    idx_all = stats.tile([L, nsp], f32, tag="idx_all")
                    spans = spans_mi
                else:
                    m_all = stats.tile([L, nfine], f32, tag="m_all" + mode)
                    spans = [(c * ch, ch) for c in range(nch)]
                pool = lpool if g0_ == 0 else lpool1
                partial = None
                for c, (lo, w) in enumerate(spans):
                    for fn in (interleave or {}).get(c, []):
                        fn()
                    if mode == "maxidx" and c == len(spans) - 1:
                        partial = partial_tournament(gp, m_all, idx_all)
                    lt = pool.tile([L, ch], f32, tag="lt")
                    ldeng = nc.scalar if c % 2 else nc.sync
                    ldeng.dma_start(out=lt[0:gp, 0:w], in_=lg_h[rows, lo:lo + w])
                    nc.gpsimd.dma_start(
                        out=lt[0:gp, 0:w], in_=gm_h[rows, lo:lo + w], accum_op=Op.add)
                    if mode == "maxidx":
                        mx8 = small.tile([L, 8], f32, tag="mx8")
                        nc.vector.max(out=mx8[0:gp, :], in_=lt[0:gp, 0:w])
                        nc.vector.tensor_copy(out=m_all[0:gp, c:c + 1], in_=mx8[0:gp, 0:1])
                        mi8 = small.tile([L, 8], u32, tag="mi8s")
                        nc.vector.max_index(out=mi8[0:gp, :], in_max=mx8[0:gp, :],
                                            in_values=lt[0:gp, 0:w])
                        nc.vector.tensor_copy(out=idx_all[0:gp, c:c + 1], in_=mi8[0:gp, 0:1])
                    else:
                        # chunk maxes at sub-chunk (sw-wide) granularity
                        lt3 = lt[0:gp, 0:w].rearrange("p (a b) -> p a b", b=sw)
                        nc.vector.reduce_max(
                            out=m_all[0:gp, c * SUB:(c + 1) * SUB], in_=lt3, axis=AX.X)
                return m_all, idx_all, partial

            def tournament(g0_, gp, m_all, rev, nslots):
                """global max + LOWEST slot attaining it (argmax tie rule)."""
                gmax = small.tile([L, 1], f32, tag="gmax")
                nc.vector.reduce_max(out=gmax[0:gp, :], in_=m_all[0:gp, :], axis=AX.X)
                sel = small.tile([L, nslots], f32, tag="sel" + str(nslots))
                nc.vector.scalar_tensor_tensor(
                    out=sel[0:gp, :], in0=m_all[0:gp, :], scalar=gmax[0:gp, 0:1],
                    in1=rev[0:gp, :], op0=Op.is_ge, op1=Op.mult)
                cmax = small.tile([L, 1], f32, tag="cmax")
                nc.vector.reduce_max(out=cmax[0:gp, :], in_=sel[0:gp, :], axis=AX.X)
                cstar = small.tile([L, 1], f32, tag="cstar")
                nc.vector.tensor_scalar(cstar[0:gp, :], cmax[0:gp, :], -1.0, float(nslots),
                                        op0=Op.mult, op1=Op.add)
                return gmax, cstar

            def finish_group(g0_, gp, hot_f, split=False):
                """hot -> W_pre gather -> tok_out rows.  split pipelines the
                gather/store pair in two row halves (shorter terminal chain)."""
                hot_i = small.tile([L, 1], i32, tag="hot_i")
                nc.vector.tensor_copy(out=hot_i[0:gp, :], in_=hot_f[0:gp, :])
                if gp == 1:
                    # single-partition indirect DMAs are rejected; pad with a
                    # zeroed second index and gather one junk row (never read)
                    nc.vector.memset(hot_i[1:2, :], 0)
                halves = [(0, gp)] if not split or gp < 32 else [
                    (0, gp // 2), (gp // 2, gp)]
                tokemb = emb.tile([L, E], f32, tag="tokemb")
                for (a, b) in halves:
                    bi = 2 if (b - a == 1 and a == 0) else b
                    nc.gpsimd.indirect_dma_start(
                        out=tokemb[a:bi, :], out_offset=None, in_=wpre_h[:],
                        in_offset=IndirectOffsetOnAxis(ap=hot_i[a:bi, 0:1], axis=0))
                steng = {"swdge": nc.gpsimd, "act": nc.scalar}.get(
                    d.get("STORE_ENG", "sp"), nc.sync)
                for (a, b) in halves:
                    steng.dma_start(out=tokout_h[g0_ + a:g0_ + b, :], in_=tokemb[a:b, :])

            def tail_rescan_start(g0_, gp, m_all):
                """Tournament (all data ready -> no DVE stalls) + issue the
                winning-sub-chunk re-read. The dependent max_index is deferred
                (tail_rescan_end) so its DMA wait never blocks the in-order
                DVE queue while the next group streams."""
                gmax, sstar = tournament(g0_, gp, m_all, frev, nfine)
                # fine-sliced row index for the re-read: (row + g0)*nfine + sstar
                ridx_f = small.tile([L, 1], f32, tag="ridx_f")
                nc.vector.tensor_tensor(out=ridx_f[0:gp, :], in0=rowb[g0_][0:gp, :],
                                        in1=sstar[0:gp, :], op=Op.add)
                ridx_i = small.tile([L, 1], i32, tag="ridx_i")
                nc.vector.tensor_copy(out=ridx_i[0:gp, :], in_=ridx_f[0:gp, :])
                gmax8 = small.tile([L, 8], f32, tag="gmax8")
                nc.vector.tensor_copy(out=gmax8[0:gp, :],
                                      in_=gmax[0:gp, 0:1].to_broadcast([gp, 8]))

                # single-partition indirect DMAs are rejected; pad the index
                # tile with a zeroed row and gather one junk row (never read)
                gpi = 2 if gp == 1 else gp
                if gpi != gp:
                    nc.vector.memset(ridx_i[gp:gpi, :], 0)

                # re-read ONLY the winning sub-chunk (logits + CCE-add gumbel);
                # bit-matches the stream pass, so gmax is found exactly
                rs = emb.tile([L, sw], f32, tag="rs")
                nc.gpsimd.indirect_dma_start(
                    out=rs[0:gpi, :], out_offset=None, in_=lg_v,
                    in_offset=IndirectOffsetOnAxis(ap=ridx_i[0:gpi, 0:1], axis=0))
                nc.gpsimd.indirect_dma_start(
                    out=rs[0:gpi, :], out_offset=None, in_=gm_v,
                    in_offset=IndirectOffsetOnAxis(ap=ridx_i[0:gpi, 0:1], axis=0),
                    compute_op=Op.add)
                return sstar, gmax8, rs

            def tail_rescan_end(g0_, gp, state):
                sstar, gmax8, rs = state
                mi8 = small.tile([L, 8], u32, tag="mi8")
                nc.vector.max_index(out=mi8[0:gp, :], in_max=gmax8[0:gp, :],
                                    in_values=rs[0:gp, :])
                mi_f = small.tile([L, 1], f32, tag="mi_f")
                nc.vector.tensor_copy(out=mi_f[0:gp, :], in_=mi8[0:gp, 0:1])
                hot_f = small.tile([L, 1], f32, tag="hot_f")
                nc.vector.scalar_tensor_tensor(
                    out=hot_f[0:gp, :], in0=sstar[0:gp, :], scalar=float(sw),
                    in1=mi_f[0:gp, :], op0=Op.mult, op1=Op.add)
                finish_group(g0_, gp, hot_f)

            def tail_maxidx(g0_, gp, m_all, idx_all, partial):
                # combine the pre-computed partial tournament with the last
                # span: the last span wins only on STRICTLY greater (argmax
                # takes the first occurrence, i.e. the earlier span on ties)
                gmaxP, hotP = partial
                lo_last = spans_mi[-1][0]
                hotL = small.tile([L, 1], f32, tag="hotL")
                nc.vector.tensor_scalar(hotL[0:gp, :], idx_all[0:gp, nsp - 1:nsp],
                                        1.0, float(lo_last), op0=Op.mult, op1=Op.add)
                gt = small.tile([L, 1], f32, tag="gtL")
                nc.vector.tensor_tensor(out=gt[0:gp, :], in0=m_all[0:gp, nsp - 1:nsp],
                                        in1=gmaxP[0:gp, :], op=Op.is_gt)
                dd = small.tile([L, 1], f32, tag="ddL")
                nc.vector.tensor_tensor(out=dd[0:gp, :], in0=hotL[0:gp, :],
                                        in1=hotP[0:gp, :], op=Op.subtract)
                hot_f = small.tile([L, 1], f32, tag="hot_f")
                nc.vector.scalar_tensor_tensor(
                    out=hot_f[0:gp, :], in0=dd[0:gp, :], scalar=gt[0:gp, 0:1],
                    in1=hotP[0:gp, :], op0=Op.mult, op1=Op.add)
                finish_group(g0_, gp, hot_f, split=d.get("WSPLIT", False))

            # psg stage 1a first: only the mask load + cold-PE matmul issue,
            # so the DVE queue stays clear for the first chunk maxes
            for t in range(2):
                psg_a(t)
            pending = {d.get("ILA", 4): [lambda: [psg_a2(t) for t in range(2)]]}
            for gi, (g0_, gp, mode) in enumerate(groups):
                last = gi == len(groups) - 1
                m_all, idx_all, partial = stream_group(g0_, gp, mode,
                                                        interleave=pending)
                pending = {}
                if not last:
                    # non-terminal group: tournament + issue the re-read now;
                    # the dependent finish and psg stages 2/3 slot a few
                    # chunks into the next group's stream (data ready by then)
                    state = tail_rescan_start(g0_, gp, m_all)
                    for t in range(2):
                        psg_b(t)
                    pending = {
                        d.get("ILB", 13): [
                            lambda g0=g0_, gp_=gp, st=state: tail_rescan_end(g0, gp_, st)],
                        d.get("ILC", 8): [lambda: [psg_c(t) for t in range(2)]],
                    }
                else:
                    if len(groups) == 1:
                        for t in range(2):
                            psg_b(t)
                    if mode == "maxidx":
                        tail_maxidx(g0_, gp, m_all, idx_all, partial)
                    else:
                        tail_rescan_end(g0_, gp, tail_rescan_start(g0_, gp, m_all))
                    if len(groups) == 1:
                        for t in range(2):
                            psg_c(t)

    return nc


_BUILD_CACHE = {}


def _get_module_v2(cap, dims_key=None, dims=None):
    key = ("v2", cap, dims_key)
    if key not in _BUILD_CACHE:
        import concourse.bacc as bacc

        nc = bacc.Bacc("TRN2", target_bir_lowering=False, debug=False)
        _build_v2(nc, cap, dims)
        nc.compile()
        _BUILD_CACHE[key] = nc
    return _BUILD_CACHE[key]


_ROWMAP_CACHE = {}


def _nearest_maps():
    """Replicate the reference's f32 grid_sample-nearest index maps with jnp
    on the same backend the reference runs on (bit-exact by construction)."""
    if "maps" not in _ROWMAP_CACHE:
        import jax.numpy as jnp

        def nearest(size):
            lin = jnp.linspace(-1.0, 1.0, size)
            ix = ((lin + 1.0) * size - 1.0) / 2.0
            return np.asarray(jnp.clip(jnp.round(ix), 0, size - 1).astype(jnp.int32))

        _ROWMAP_CACHE["maps"] = (nearest(V), nearest(E))
    return _ROWMAP_CACHE["maps"]


_TRI = None

# test/dev hooks: set TRACE=True before calling kernel() to capture an NTFF
# profile; the BassKernelResults of the last run is stored in LAST_RESULT.
TRACE = False
LAST_RESULT = None
LAST_MODULE = None


def kernel(logits, rwrt_attention_mask, psg_input_ids, word_embeddings, gumbel_noise):
    from concourse.bass_utils import run_bass_kernel_spmd

    global _TRI, LAST_RESULT, LAST_MODULE
    logits = np.ascontiguousarray(np.asarray(logits, dtype=np.float32))
    gumbel = np.ascontiguousarray(np.asarray(gumbel_noise, dtype=np.float32))
    mask = np.ascontiguousarray(np.asarray(rwrt_attention_mask, dtype=np.int32))
    psg = np.ascontiguousarray(np.asarray(psg_input_ids, dtype=np.int32))
    wte = np.ascontiguousarray(np.asarray(word_embeddings, dtype=np.float32))

    rowmap, colmap = _nearest_maps()
    # token branch reads W[rowmap][:, colmap]; constant-fold both static maps
    wpre = np.ascontiguousarray(wte[rowmap][:, colmap])
    if _TRI is None:
        _TRI = np.ascontiguousarray(np.triu(np.ones((L, L), dtype=np.float32)))

    # ---- sharding: compact the active (mask==1) tokens across cores ----
    flat_mask = mask.reshape(B * L) != 0
    ids = np.nonzero(flat_mask)[0].astype(np.int64)    # batch-major order
    n_act = int(ids.size)
    cap = max(8, -(-n_act // N_CORES))                 # tokens per core
    cap += cap & 1     # even row counts only: an odd-cap module (odd-row
                       # groups / odd-row tok_out) hit NRT_EXEC_UNIT_
                       # UNRECOVERABLE on device; one pad slot is free
    pad_id = int(ids[-1]) if n_act else 0
    ids_p = np.full(N_CORES * cap, pad_id, dtype=np.int64)
    ids_p[:n_act] = ids
    lg_flat = logits.reshape(B * L, V)
    gm_flat = gumbel.reshape(B * L, V)

    nc = _get_module_v2(cap)
    LAST_MODULE = nc

    in_maps = []
    for m in range(N_CORES):
        sl = ids_p[m * cap:(m + 1) * cap]
        im = {
            "lg": np.ascontiguousarray(lg_flat[sl]),
            "gm": np.ascontiguousarray(gm_flat[sl]),
            "mask": mask[2 * m:2 * m + 2].reshape(2 * L, 1),
            "psg": psg[2 * m:2 * m + 2].reshape(2 * L, 1),
            "wte": wte,
            "wpre": wpre,
            "tri": _TRI,
        }
        in_maps.append(im)

    try:
        LAST_RESULT = run_bass_kernel_spmd(nc, in_maps, list(range(N_CORES)), trace=TRACE)
    except Exception:
        # the axon-relayed device occasionally reports a transient
        # NRT_EXEC_UNIT_UNRECOVERABLE; re-runs (with backoff) recover it
        import time as _time

        for backoff in (2.0, 15.0):
            _time.sleep(backoff)
            try:
                LAST_RESULT = run_bass_kernel_spmd(
                    nc, in_maps, list(range(N_CORES)), trace=TRACE)
                break
            except Exception:
                if backoff == 15.0:
                    raise
    res = LAST_RESULT.results

    # ---- unshard: psg partial everywhere + scatter-add the token rows ----
    out = np.concatenate(
        [res[m]["psg_out"].reshape(2, L, E) for m in range(N_CORES)], axis=0)
    if n_act:
        tok = np.concatenate(
            [res[m]["tok_out"].reshape(cap, E) for m in range(N_CORES)], axis=0)
        flat = out.reshape(B * L, E)
        flat[ids] += tok[:n_act]
    return out
